# revision 16
# baseline (speedup 1.0000x reference)
"""GCN (3x GCNConv + global max pool + FC + log_softmax) on 8 Trainium2 NeuronCores.

v2 — pipelined rewrite of the working baseline:
  - 1D node partition: core c owns rows [12500c, 12500(c+1)).
  - dinv folded into inputs: x_pre = dinv*x on host; each conv's post is a single
    Act-engine relu with per-partition scale (dinv^2 for inner convs, dinv for the
    last) since relu(c*y) = c*relu(y) for c>0 and D(AW) = (DA)W.
  - dense(l): hs = a_pre @ W_l per local tile (PE, bf16) -> hs tile -> bounce DRAM.
    Quarter-granular AllGather (4 per layer, Shared outputs) so collectives start
    as soon as a quarter of the dense outputs lands and overlap with aggregation.
  - Aggregation: per-edge gather of table rows via gpsimd dma_gather (int16 idxs,
    4 src chunks = the AllGather quarters), then segment-sum via PE matmuls with
    per-128-edge one-hot selection matrices built on DVE, accumulated in PSUM per
    128-dst tile.  Gather idxs are src-sorted within groups; padding slots are -1
    (skipped by the DMA ucode).
  - dense(l+1) tiles are emitted immediately after each agg(l) tile so layers
    pipeline; AllGather triggers sit between layers' gather streams on gpsimd.
  - Pooling (segment max over graphs), tiny FC and log_softmax run on host.
"""

import sys

sys.path.insert(0, "/opt/trn_rl_repo")

import numpy as np
import ml_dtypes

import concourse.bass as bass
import concourse.bacc as bacc
import concourse.tile as tile
from concourse import mybir
from concourse.bass_utils import run_bass_kernel_spmd
from concourse.masks import make_identity

P = 128
N_NODES = 100000
N_EDGES = 1600000
N_GRAPHS = 64
N_CORES = 8
NODES_PER_CORE = N_NODES // N_CORES          # 12500
NTILES = (NODES_PER_CORE + P - 1) // P       # 98 (last tile 84 rows)
LAST_ROWS = NODES_PER_CORE - (NTILES - 1) * P  # 84
NCHUNK = 4
QT = [25, 25, 25, 23]                        # quarter tile counts
Q_TSTART = [0, 25, 50, 75]
Q_ROWS = [3200, 3200, 3200, 2900]            # per-core rows per quarter
Q_RSTART = [0, 3200, 6400, 9600]
CH_ROWS = [r * N_CORES for r in Q_ROWS]      # chunk rows: 25600,25600,25600,23200
WT = 8                                       # dst tiles per wave
F0 = 512
FW = 128                                     # table width (padded)
NCOLS_MM = [128, 128, 64]                    # agg matmul N per layer
MAX_CALL_BLOCKS = 47                         # ~6K idxs per dma_gather call
NQUEUES = 4
DMA_SCRATCH = 16384
dt = mybir.dt
BF = ml_dtypes.bfloat16


def _rows(t):
    return LAST_ROWS if t == NTILES - 1 else P


def _host_prep(edge_index):
    """Build the shared (cross-core) aggregation schedule + per-core index data."""
    src = np.concatenate([edge_index[0], np.arange(N_NODES, dtype=np.int64)])
    dst = np.concatenate([edge_index[1], np.arange(N_NODES, dtype=np.int64)])
    deg = np.bincount(dst, minlength=N_NODES).astype(np.float32)
    dinv = (1.0 / np.sqrt(deg)).astype(np.float32)

    waves = [list(range(w, min(w + WT, NTILES))) for w in range(0, NTILES, WT)]

    # src -> (chunk, row-in-chunk): chunk = quarter of the owner's tile range
    s_core = src // NODES_PER_CORE
    s_loc = src % NODES_PER_CORE
    s_t = s_loc // P
    qs = np.searchsorted(np.array([25, 50, 75]), s_t, side="right")  # quarter id
    q_rows = np.array(Q_ROWS)
    q_rstart = np.array(Q_RSTART)
    s_row = s_core * q_rows[qs] + (s_loc - q_rstart[qs])

    per_core = []
    cnts = np.zeros((N_CORES, NTILES, NCHUNK), np.int64)
    d_core = dst // NODES_PER_CORE
    for c in range(N_CORES):
        m = d_core == c
        srow, sq = s_row[m], qs[m]
        dl = dst[m] - c * NODES_PER_CORE
        t = dl // P
        key = t * NCHUNK + sq
        o = np.lexsort((srow, key))          # group by (t,chunk), src-sorted within
        srow, dl, key = srow[o], dl[o], key[o]
        cnt = np.bincount(key, minlength=NTILES * NCHUNK).reshape(NTILES, NCHUNK)
        cnts[c] = cnt
        per_core.append((srow, dl, cnt))

    blocks = np.maximum((cnts.max(axis=0) + P - 1) // P, 1)  # [NTILES, NCHUNK]

    S_ch = [int(blocks[:, ch].sum() * P) for ch in range(NCHUNK)]
    NB_total = int(blocks.sum())

    # idx slot order per chunk: (wave, tile, block); didx col order: (wave, tile, chunk, block)
    chunk_start = [dict() for _ in range(NCHUNK)]
    for ch in range(NCHUNK):
        pos = 0
        for wtiles in waves:
            for t in wtiles:
                chunk_start[ch][t] = pos
                pos += int(blocks[t, ch]) * P
    g_start = {}
    g = 0
    for wtiles in waves:
        for t in wtiles:
            for ch in range(NCHUNK):
                g_start[(t, ch)] = g
                g += int(blocks[t, ch])
    assert g == NB_total

    idx16 = [np.zeros((N_CORES, P, S_ch[ch] // 16), np.int16) for ch in range(NCHUNK)]
    didx = np.full((N_CORES, P, NB_total), -1.0, np.float32)

    for c in range(N_CORES):
        srow, dl, cnt = per_core[c]
        ends = np.cumsum(cnt.reshape(-1))
        starts = ends - cnt.reshape(-1)
        idx_slots = [np.zeros(S_ch[ch], np.int16) for ch in range(NCHUNK)]
        didx_slots = np.full(NB_total * P, -1.0, np.float32)
        for t in range(NTILES):
            for ch in range(NCHUNK):
                k = t * NCHUNK + ch
                n = cnt[t, ch]
                if n == 0:
                    continue
                e0, e1 = starts[k], ends[k]
                cs = chunk_start[ch][t]
                idx_slots[ch][cs:cs + n] = srow[e0:e1].astype(np.int16)
                gs = g_start[(t, ch)] * P
                didx_slots[gs:gs + n] = (dl[e0:e1] - t * P).astype(np.float32)
        for ch in range(NCHUNK):
            w16 = idx_slots[ch].reshape(-1, 16).T  # [16, S/16]
            idx16[ch][c] = np.tile(w16, (8, 1))
        didx[c] = didx_slots.reshape(-1, P).T

    max_nb = max(
        int(sum(blocks[t, ch] for t in wtiles))
        for wtiles in waves for ch in range(NCHUNK)
    )
    meta = {
        "waves": waves,
        "blocks": blocks,
        "S_ch": S_ch,
        "NB_total": NB_total,
        "chunk_start": chunk_start,
        "g_start": g_start,
        "max_nb": max_nb,
    }
    return dinv, idx16, didx, meta


def _build_program(meta, has_bias):
    waves = meta["waves"]
    blocks = meta["blocks"]
    S_ch = meta["S_ch"]
    NB_total = meta["NB_total"]
    chunk_start = meta["chunk_start"]
    g_start = meta["g_start"]
    max_nb = meta["max_nb"]

    nc = bacc.Bacc(
        "TRN2", target_bir_lowering=False, debug=False, num_devices=N_CORES,
        num_swdge_queues=NQUEUES, dynamic_dma_scratch_size=DMA_SCRATCH,
    )

    xT_io = nc.dram_tensor("xT", [F0, NODES_PER_CORE], dt.bfloat16, kind="ExternalInput").ap()
    sc_in_io = nc.dram_tensor("scIn", [P, NTILES], dt.float32, kind="ExternalInput").ap()   # dinv^2
    sc_out_io = nc.dram_tensor("scOut", [P, NTILES], dt.float32, kind="ExternalInput").ap() # dinv
    w1_io = nc.dram_tensor("W1sb", [P, F0], dt.bfloat16, kind="ExternalInput").ap()
    w2_io = nc.dram_tensor("W2pad", [P, P], dt.bfloat16, kind="ExternalInput").ap()
    w3_io = nc.dram_tensor("W3pad", [P, P], dt.bfloat16, kind="ExternalInput").ap()
    iota_io = nc.dram_tensor("iota", [P, P], dt.bfloat16, kind="ExternalInput").ap()
    idx_ios = [
        nc.dram_tensor(f"idx{ch}", [P, S_ch[ch] // 16], dt.int16, kind="ExternalInput").ap()
        for ch in range(NCHUNK)
    ]
    didx_io = nc.dram_tensor("didx", [P, NB_total], dt.bfloat16, kind="ExternalInput").ap()
    out_io = nc.dram_tensor("out3", [NODES_PER_CORE, 64], dt.float32, kind="ExternalOutput").ap()

    with tile.TileContext(nc) as tc:
        with (
            tc.tile_pool(name="const", bufs=1) as constp,
            tc.tile_pool(name="aT", bufs=1) as aTp,
            tc.tile_pool(name="hs", bufs=8) as hsp,
            tc.tile_pool(name="psum_d", bufs=2, space="PSUM") as psdp,
            tc.tile_pool(name="psum_a", bufs=3, space="PSUM") as psap,
            tc.tile_pool(name="psum_t", bufs=2, space="PSUM") as pstp,
            tc.tile_pool(name="dram", bufs=1, space="DRAM") as dramp,
        ):
            # ---- constants ----
            iota_t = constp.tile([P, P], dt.bfloat16)
            nc.sync.dma_start(iota_t[:], iota_io[:])
            sc_in = constp.tile([P, NTILES], dt.float32)
            nc.sync.dma_start(sc_in[:], sc_in_io[:])
            sc_out = constp.tile([P, NTILES], dt.float32)
            nc.sync.dma_start(sc_out[:], sc_out_io[:])
            w1_sb = constp.tile([P, F0], dt.bfloat16)
            nc.sync.dma_start(w1_sb[:], w1_io[:])
            w2_sb = constp.tile([P, P], dt.bfloat16)
            nc.sync.dma_start(w2_sb[:], w2_io[:])
            w3_sb = constp.tile([P, P], dt.bfloat16)
            nc.sync.dma_start(w3_sb[:], w3_io[:])
            didx_sb = constp.tile([P, NB_total], dt.bfloat16)
            nc.sync.dma_start(didx_sb[:], didx_io[:])
            identb = constp.tile([P, P], dt.bfloat16)
            make_identity(nc, identb[:])

            # a_preT per layer per quarter:  [128 feats, qrows]
            aTq = {
                l: [aTp.tile([P, Q_ROWS[q] ], dt.bfloat16, tag=f"a{l}q{q}", name=f"a{l}q{q}")
                    for q in range(4)]
                for l in (1, 2)
            }

            bounces = [
                dramp.tile([NODES_PER_CORE, FW], dt.bfloat16, tag=f"bnc{l}", name=f"bnc{l}")
                for l in range(3)
            ]
            tbls = [
                [dramp.tile([CH_ROWS[q], FW], dt.bfloat16, tag=f"tbl{l}q{q}",
                            name=f"tbl{l}q{q}", addr_space="Shared") for q in range(4)]
                for l in range(3)
            ]

            def q_of_t(t):
                return min(t // 25, 3)

            def dense_tile(l, t):
                """hs tile = a_pre @ W  (psum) -> bf16 hs tile -> bounce DRAM."""
                r = _rows(t)
                c0 = t * P
                ps = psdp.tile([P, P], dt.float32, space="PSUM", tag="pd")
                if l == 0:
                    h = 0 if t < 49 else 1
                    hb = 0 if t < 49 else 49 * P
                    nk = F0 // P
                    for k in range(nk):
                        xk = xhalves[h][k]
                        nc.tensor.matmul(
                            out=ps[:r, :], lhsT=xk[:, c0 - hb:c0 - hb + r],
                            rhs=w1_sb[:, k * P:(k + 1) * P],
                            start=(k == 0), stop=(k == nk - 1),
                        )
                else:
                    q = q_of_t(t)
                    off = (t - Q_TSTART[q]) * P
                    w = w2_sb if l == 1 else w3_sb
                    nc.tensor.matmul(
                        out=ps[:r, :], lhsT=aTq[l][q][:, off:off + r], rhs=w[:],
                        start=True, stop=True,
                    )
                hs = hsp.tile([P, P], dt.bfloat16, tag="hs")
                nc.vector.tensor_copy(hs[:r, :], ps[:r, :])
                nc.sync.dma_start(bounces[l][c0:c0 + r, :], hs[:r, :])

            def emit_allgathers(l):
                for q in range(4):
                    r0 = Q_RSTART[q]
                    nc.gpsimd.collective_compute(
                        "AllGather", mybir.AluOpType.bypass,
                        replica_groups=[list(range(N_CORES))],
                        ins=[bounces[l][r0:r0 + Q_ROWS[q], :]],
                        outs=[tbls[l][q][:]],
                    )

            # ---- dense layer 0: staged x stripes, two node-halves ----
            xhalves = []
            with tc.tile_pool(name="xs", bufs=5) as xsp:
                for h in range(2):
                    hb = 0 if h == 0 else 49 * P
                    hc = 49 * P if h == 0 else NODES_PER_CORE - 49 * P
                    ks = []
                    for k in range(F0 // P):
                        xk = xsp.tile([P, 49 * P], dt.bfloat16, tag="xh")
                        nc.sync.dma_start(
                            xk[:, :hc], xT_io[k * P:(k + 1) * P, hb:hb + hc]
                        )
                        ks.append(xk)
                    xhalves.append(ks)
                for t in range(NTILES):
                    dense_tile(0, t)
                emit_allgathers(0)

            def agg_layer(l, idxp, msgp, selp, workp):
                ncols = NCOLS_MM[l]
                for w, wtiles in enumerate(waves):
                    # gathers (one call per chunk, queue = chunk)
                    msg_tiles = {}
                    for ch in range(NCHUNK):
                        nb = int(sum(blocks[t, ch] for t in wtiles))
                        s0 = chunk_start[ch][wtiles[0]]
                        S = nb * P
                        iw = idxp.tile([P, max_nb * P // 16], dt.int16, tag="idx")
                        nc.sync.dma_start(iw[:, :S // 16], idx_ios[ch][:, s0 // 16:(s0 + S) // 16])
                        mt = msgp.tile([P, max_nb, FW], dt.bfloat16, tag="msg")
                        for b0 in range(0, nb, MAX_CALL_BLOCKS):
                            b1 = min(b0 + MAX_CALL_BLOCKS, nb)
                            Ssub = (b1 - b0) * P
                            nc.gpsimd.dma_gather(
                                out_ap=mt[:, b0:b1, :],
                                in_ap=tbls[l][ch][:],
                                idxs_ap=iw[:, b0 * P // 16:b1 * P // 16],
                                num_idxs=Ssub, num_idxs_reg=Ssub,
                                elem_size=FW, elem_step=FW,
                                single_packet=False,
                                queue_num=ch % NQUEUES,
                            )
                        msg_tiles[ch] = mt

                    # selection matrices, batches of 8 blocks in didx col order
                    gw0 = g_start[(wtiles[0], 0)]
                    gw1 = gw0 + int(sum(blocks[t, ch] for t in wtiles for ch in range(NCHUNK)))
                    BB = 8
                    sel_tiles = {}
                    for q0 in range(gw0, gw1, BB):
                        q1 = min(q0 + BB, gw1)
                        st = selp.tile([P, BB, P], dt.bfloat16, tag="sel")
                        nc.vector.tensor_tensor(
                            out=st[:, :q1 - q0, :],
                            in0=didx_sb[:, q0:q1, None].to_broadcast([P, q1 - q0, P]),
                            in1=iota_t[:, None, :].to_broadcast([P, q1 - q0, P]),
                            op=mybir.AluOpType.is_equal,
                        )
                        for q in range(q0, q1):
                            sel_tiles[q] = (st, q - q0)

                    # per-tile matmul accumulation + post + (dense l+1)
                    for ti, t in enumerate(wtiles):
                        r = _rows(t)
                        c0 = t * P
                        ps = psap.tile([P, P], dt.float32, space="PSUM", tag="pa")
                        mms = []
                        for ch in range(NCHUNK):
                            coff = int(sum(blocks[tt, ch] for tt in wtiles[:ti]))
                            for b in range(int(blocks[t, ch])):
                                mms.append((g_start[(t, ch)] + b, ch, coff + b))
                        for i, (q, ch, col) in enumerate(mms):
                            st, j = sel_tiles[q]
                            nc.tensor.matmul(
                                out=ps[:r, :ncols], lhsT=st[:, j, :r],
                                rhs=msg_tiles[ch][:, col, :ncols],
                                start=(i == 0),
                                stop=(i == len(mms) - 1),
                            )
                        if l < 2:
                            outt = workp.tile([P, P], dt.bfloat16, tag="outb")
                            nc.scalar.activation(
                                outt[:r, :], ps[:r, :],
                                mybir.ActivationFunctionType.Relu,
                                scale=sc_in[:r, t:t + 1],
                            )
                            pst = pstp.tile([P, P], dt.bfloat16, space="PSUM", tag="pt")
                            nc.tensor.transpose(
                                out=pst[:, :r], in_=outt[:r, :], identity=identb[:r, :r]
                            )
                            q = q_of_t(t)
                            off = (t - Q_TSTART[q]) * P
                            nc.scalar.activation(
                                aTq[l + 1][q][:, off:off + r], pst[:, :r],
                                mybir.ActivationFunctionType.Copy,
                            )
                            dense_tile(l + 1, t)
                        else:
                            outt = workp.tile([P, 64], dt.float32, tag="outf")
                            nc.scalar.activation(
                                outt[:r, :], ps[:r, :64],
                                mybir.ActivationFunctionType.Relu,
                                scale=sc_out[:r, t:t + 1],
                            )
                            nc.sync.dma_start(out_io[c0:c0 + r, :], outt[:r, :])
                if l < 2:
                    emit_allgathers(l + 1)

            with (
                tc.tile_pool(name="idxw", bufs=8) as idxp,
                tc.tile_pool(name="msgs", bufs=6) as msgp,
                tc.tile_pool(name="sel", bufs=12) as selp,
                tc.tile_pool(name="work", bufs=6) as workp,
            ):
                agg_layer(0, idxp, msgp, selp, workp)
                agg_layer(1, idxp, msgp, selp, workp)
                agg_layer(2, idxp, msgp, selp, workp)

    nc.compile()
    return nc


def _pack_inputs(x, dinv, W1, W2, W3, idx16, didx):
    iota_rep = np.tile(np.arange(P, dtype=np.float32)[None, :], (P, 1)).astype(BF)
    w1sb = np.zeros((P, F0), np.float32)
    for k in range(F0 // P):
        w1sb[:, k * P:(k + 1) * P] = W1[k * P:(k + 1) * P, :]
    w2pad = np.zeros((P, P), np.float32)
    w2pad[:, :64] = W2
    w3pad = np.zeros((P, P), np.float32)
    w3pad[:64, :32] = W3

    in_maps = []
    for c in range(N_CORES):
        lo = c * NODES_PER_CORE
        dv = dinv[lo:lo + NODES_PER_CORE]
        xs = x[lo:lo + NODES_PER_CORE].astype(np.float32) * dv[:, None]
        sc_in = np.ones((P, NTILES), np.float32)   # dinv^2 per tile col
        sc_out = np.ones((P, NTILES), np.float32)  # dinv per tile col
        for t in range(NTILES):
            r = _rows(t)
            sc_in[:r, t] = dv[t * P:t * P + r] ** 2
            sc_out[:r, t] = dv[t * P:t * P + r]
        in_maps.append({
            "xT": np.ascontiguousarray(xs.T).astype(BF),
            "scIn": sc_in,
            "scOut": sc_out,
            "W1sb": w1sb.astype(BF),
            "W2pad": w2pad.astype(BF),
            "W3pad": w3pad.astype(BF),
            "iota": iota_rep,
            **{f"idx{ch}": idx16[ch][c] for ch in range(NCHUNK)},
            "didx": didx[c].astype(BF),
        })
    return in_maps


_TRACE = [False]          # set by test harness to request a profiled run
_LAST_RESULT = [None]     # BassKernelResults of the last run (for profiling)


def kernel(x, edge_index, batch, W1, b1, W2, b2, W3, b3, Wfc, bfc):
    x = np.asarray(x)
    edge_index = np.asarray(edge_index)
    batch = np.asarray(batch)
    W1, b1 = np.asarray(W1), np.asarray(b1)
    W2, b2 = np.asarray(W2), np.asarray(b2)
    W3, b3 = np.asarray(W3), np.asarray(b3)
    Wfc, bfc = np.asarray(Wfc), np.asarray(bfc)
    has_bias = bool(np.abs(b1).max() or np.abs(b2).max() or np.abs(b3).max())
    assert not has_bias, "nonzero conv biases not wired up in v2"

    dinv, idx16, didx, meta = _host_prep(edge_index.astype(np.int64))
    nc = _build_program(meta, has_bias)
    in_maps = _pack_inputs(x, dinv, W1, W2, W3, idx16, didx)
    res = run_bass_kernel_spmd(
        nc, in_maps, core_ids=list(range(N_CORES)), trace=_TRACE[0]
    )
    _LAST_RESULT[0] = res

    h3 = np.concatenate([res.results[c]["out3"][:, :32] for c in range(N_CORES)], axis=0)

    # host epilogue: segment max pool + FC + log_softmax (float64 for stability)
    pooled = np.full((N_GRAPHS, 32), -np.inf, np.float64)
    bnd = np.searchsorted(batch, np.arange(N_GRAPHS + 1))
    for g in range(N_GRAPHS):
        if bnd[g + 1] > bnd[g]:
            pooled[g] = h3[bnd[g]:bnd[g + 1]].max(axis=0)
    logits = pooled @ Wfc.astype(np.float64) + bfc.astype(np.float64)
    m = logits.max(axis=1, keepdims=True)
    lse = m + np.log(np.exp(logits - m).sum(axis=1, keepdims=True))
    return (logits - lse).astype(np.float32)


# revision 28
# speedup vs baseline: 1.2726x; 1.2726x over previous
"""GCN (3x GCNConv + global max pool + FC + log_softmax) on 8 Trainium2 NeuronCores.

v2 — pipelined rewrite of the working baseline:
  - 1D node partition: core c owns rows [12500c, 12500(c+1)).
  - dinv folded into inputs: x_pre = dinv*x on host; each conv's post is a single
    Act-engine relu with per-partition scale (dinv^2 for inner convs, dinv for the
    last) since relu(c*y) = c*relu(y) for c>0 and D(AW) = (DA)W.
  - dense(l): hs = a_pre @ W_l per local tile (PE, bf16) -> hs tile -> bounce DRAM.
    Quarter-granular AllGather (4 per layer, Shared outputs) so collectives start
    as soon as a quarter of the dense outputs lands and overlap with aggregation.
  - Aggregation: per-edge gather of table rows via gpsimd dma_gather (int16 idxs,
    4 src chunks = the AllGather quarters), then segment-sum via PE matmuls with
    per-128-edge one-hot selection matrices built on DVE, accumulated in PSUM per
    128-dst tile.  Gather idxs are src-sorted within groups; padding slots are -1
    (skipped by the DMA ucode).
  - dense(l+1) tiles are emitted immediately after each agg(l) tile so layers
    pipeline; AllGather triggers sit between layers' gather streams on gpsimd.
  - Pooling (segment max over graphs), tiny FC and log_softmax run on host.
"""

import sys

sys.path.insert(0, "/opt/trn_rl_repo")

import numpy as np
import ml_dtypes

import concourse.bass as bass
import concourse.bacc as bacc
import concourse.tile as tile
from concourse import mybir
from concourse.bass_utils import run_bass_kernel_spmd
from concourse.masks import make_identity

P = 128
N_NODES = 100000
N_EDGES = 1600000
N_GRAPHS = 64
N_CORES = 8
NODES_PER_CORE = N_NODES // N_CORES          # 12500
NTILES = (NODES_PER_CORE + P - 1) // P       # 98 (last tile 84 rows)
LAST_ROWS = NODES_PER_CORE - (NTILES - 1) * P  # 84
NCHUNK = 4
QT = [25, 25, 25, 23]                        # quarter tile counts
Q_TSTART = [0, 25, 50, 75]
Q_ROWS = [3200, 3200, 3200, 2900]            # per-core rows per quarter
Q_RSTART = [0, 3200, 6400, 9600]
CH_ROWS = [r * N_CORES for r in Q_ROWS]      # chunk rows: 25600,25600,25600,23200
WT = 8                                       # dst tiles per wave
F0 = 512
FW = 128                                     # table width (padded)
NCOLS_MM = [128, 128, 64]                    # agg matmul N per layer
MAX_CALL_BLOCKS = 47                         # ~6K idxs per dma_gather call
BB = 16                                      # sel blocks built per DVE instruction
NQUEUES = 4
DMA_SCRATCH = 16384
dt = mybir.dt
BF = ml_dtypes.bfloat16


def _rows(t):
    return LAST_ROWS if t == NTILES - 1 else P


def _host_prep(edge_index):
    """Build the shared (cross-core) aggregation schedule + per-core index data."""
    src = np.concatenate([edge_index[0], np.arange(N_NODES, dtype=np.int64)])
    dst = np.concatenate([edge_index[1], np.arange(N_NODES, dtype=np.int64)])
    deg = np.bincount(dst, minlength=N_NODES).astype(np.float32)
    dinv = (1.0 / np.sqrt(deg)).astype(np.float32)

    waves = [list(range(w, min(w + WT, NTILES))) for w in range(0, NTILES, WT)]

    # src -> (chunk, row-in-chunk): chunk = quarter of the owner's tile range
    s_core = src // NODES_PER_CORE
    s_loc = src % NODES_PER_CORE
    s_t = s_loc // P
    qs = np.searchsorted(np.array([25, 50, 75]), s_t, side="right")  # quarter id
    q_rows = np.array(Q_ROWS)
    q_rstart = np.array(Q_RSTART)
    s_row = s_core * q_rows[qs] + (s_loc - q_rstart[qs])

    per_core = []
    cnts = np.zeros((N_CORES, NTILES, NCHUNK), np.int64)
    d_core = dst // NODES_PER_CORE
    for c in range(N_CORES):
        m = d_core == c
        srow, sq = s_row[m], qs[m]
        dl = dst[m] - c * NODES_PER_CORE
        t = dl // P
        key = t * NCHUNK + sq
        o = np.lexsort((srow, key))          # group by (t,chunk), src-sorted within
        srow, dl, key = srow[o], dl[o], key[o]
        cnt = np.bincount(key, minlength=NTILES * NCHUNK).reshape(NTILES, NCHUNK)
        cnts[c] = cnt
        per_core.append((srow, dl, cnt))

    blocks = np.maximum((cnts.max(axis=0) + P - 1) // P, 1)  # [NTILES, NCHUNK]

    S_ch = [int(blocks[:, ch].sum() * P) for ch in range(NCHUNK)]
    NB_total = int(blocks.sum())

    # idx slot order per chunk: (wave, tile, block); didx col order: (wave, tile, chunk, block)
    chunk_start = [dict() for _ in range(NCHUNK)]
    for ch in range(NCHUNK):
        pos = 0
        for wtiles in waves:
            for t in wtiles:
                chunk_start[ch][t] = pos
                pos += int(blocks[t, ch]) * P
    g_start = {}
    g = 0
    for wtiles in waves:
        for t in wtiles:
            for ch in range(NCHUNK):
                g_start[(t, ch)] = g
                g += int(blocks[t, ch])
    assert g == NB_total

    idx16 = [np.zeros((N_CORES, P, S_ch[ch] // 16), np.int16) for ch in range(NCHUNK)]
    didx = np.full((N_CORES, P, NB_total), -1.0, np.float32)

    for c in range(N_CORES):
        srow, dl, cnt = per_core[c]
        ends = np.cumsum(cnt.reshape(-1))
        starts = ends - cnt.reshape(-1)
        idx_slots = [np.zeros(S_ch[ch], np.int16) for ch in range(NCHUNK)]
        didx_slots = np.full(NB_total * P, -1.0, np.float32)
        for t in range(NTILES):
            for ch in range(NCHUNK):
                k = t * NCHUNK + ch
                n = cnt[t, ch]
                if n == 0:
                    continue
                e0, e1 = starts[k], ends[k]
                cs = chunk_start[ch][t]
                idx_slots[ch][cs:cs + n] = srow[e0:e1].astype(np.int16)
                gs = g_start[(t, ch)] * P
                didx_slots[gs:gs + n] = (dl[e0:e1] - t * P).astype(np.float32)
        for ch in range(NCHUNK):
            w16 = idx_slots[ch].reshape(-1, 16).T  # [16, S/16]
            idx16[ch][c] = np.tile(w16, (8, 1))
        didx[c] = didx_slots.reshape(-1, P).T

    max_nb = max(
        int(sum(blocks[t, ch] for t in wtiles))
        for wtiles in waves for ch in range(NCHUNK)
    )
    meta = {
        "waves": waves,
        "blocks": blocks,
        "S_ch": S_ch,
        "NB_total": NB_total,
        "chunk_start": chunk_start,
        "g_start": g_start,
        "max_nb": max_nb,
    }
    return dinv, idx16, didx, meta


def _build_program(meta, has_bias):
    waves = meta["waves"]
    blocks = meta["blocks"]
    S_ch = meta["S_ch"]
    NB_total = meta["NB_total"]
    chunk_start = meta["chunk_start"]
    g_start = meta["g_start"]
    max_nb = meta["max_nb"]

    nc = bacc.Bacc(
        "TRN2", target_bir_lowering=False, debug=False, num_devices=N_CORES,
        num_swdge_queues=NQUEUES, dynamic_dma_scratch_size=DMA_SCRATCH,
    )

    xT_io = nc.dram_tensor("xT", [F0, NODES_PER_CORE], dt.bfloat16, kind="ExternalInput").ap()
    sc_in_io = nc.dram_tensor("scIn", [P, NTILES], dt.float32, kind="ExternalInput").ap()   # dinv^2
    sc_out_io = nc.dram_tensor("scOut", [P, NTILES], dt.float32, kind="ExternalInput").ap() # dinv
    w1_io = nc.dram_tensor("W1sb", [P, F0], dt.bfloat16, kind="ExternalInput").ap()
    w2_io = nc.dram_tensor("W2pad", [P, P], dt.bfloat16, kind="ExternalInput").ap()
    w3_io = nc.dram_tensor("W3pad", [P, P], dt.bfloat16, kind="ExternalInput").ap()
    iota_io = nc.dram_tensor("iota", [P, P], dt.bfloat16, kind="ExternalInput").ap()
    iotaR_io = nc.dram_tensor("iotaR", [P, P * BB], dt.bfloat16, kind="ExternalInput").ap()
    idx_ios = [
        nc.dram_tensor(f"idx{ch}", [P, S_ch[ch] // 16], dt.int16, kind="ExternalInput").ap()
        for ch in range(NCHUNK)
    ]
    didx_io = nc.dram_tensor("didx", [P, NB_total], dt.bfloat16, kind="ExternalInput").ap()
    out_io = nc.dram_tensor("out3", [NODES_PER_CORE, 64], dt.float32, kind="ExternalOutput").ap()

    with tile.TileContext(nc) as tc:
        with (
            tc.tile_pool(name="const", bufs=1) as constp,
            tc.tile_pool(name="aT", bufs=1) as aTp,
            tc.tile_pool(name="hs", bufs=8) as hsp,
            tc.tile_pool(name="psum_d", bufs=2, space="PSUM") as psdp,
            tc.tile_pool(name="psum_a", bufs=4, space="PSUM") as psap,
            tc.tile_pool(name="psum_t", bufs=2, space="PSUM") as pstp,
            tc.tile_pool(name="dram", bufs=1, space="DRAM") as dramp,
        ):
            # ---- constants ----
            iota_t = constp.tile([P, P], dt.bfloat16)
            nc.sync.dma_start(iota_t[:], iota_io[:])
            sc_in = constp.tile([P, NTILES], dt.float32)
            nc.sync.dma_start(sc_in[:], sc_in_io[:])
            sc_out = constp.tile([P, NTILES], dt.float32)
            nc.sync.dma_start(sc_out[:], sc_out_io[:])
            w1_sb = constp.tile([P, F0], dt.bfloat16)
            nc.sync.dma_start(w1_sb[:], w1_io[:])
            w2_sb = constp.tile([P, P], dt.bfloat16)
            nc.sync.dma_start(w2_sb[:], w2_io[:])
            w3_sb = constp.tile([P, P], dt.bfloat16)
            nc.sync.dma_start(w3_sb[:], w3_io[:])
            didx_sb = constp.tile([P, NB_total], dt.bfloat16)
            nc.sync.dma_start(didx_sb[:], didx_io[:])
            iotaR = constp.tile([P, P, BB], dt.bfloat16)   # [p, j, g] = j
            nc.sync.dma_start(iotaR[:], iotaR_io[:])
            identb = constp.tile([P, P], dt.bfloat16)
            make_identity(nc, identb[:])

            # a_preT tiles (one set, reallocated per layer): [128 feats, qrows]
            current_aT = {}

            bounces = [
                dramp.tile([NODES_PER_CORE, FW], dt.bfloat16, tag=f"bnc{l}", name=f"bnc{l}")
                for l in range(3)
            ]
            tbls = [
                [dramp.tile([CH_ROWS[q], FW], dt.bfloat16, tag=f"tbl{l}q{q}",
                            name=f"tbl{l}q{q}", addr_space="Shared") for q in range(4)]
                for l in range(3)
            ]

            def q_of_t(t):
                return min(t // 25, 3)

            def dense_tile(l, t):
                """hs tile = a_pre @ W  (psum) -> bf16 hs tile -> bounce DRAM."""
                r = _rows(t)
                c0 = t * P
                ps = psdp.tile([P, P], dt.float32, space="PSUM", tag="pd")
                if l == 0:
                    h = 0 if t < 49 else 1
                    hb = 0 if t < 49 else 49 * P
                    nk = F0 // P
                    for k in range(nk):
                        xk = xhalves[h][k]
                        nc.tensor.matmul(
                            out=ps[:r, :], lhsT=xk[:, c0 - hb:c0 - hb + r],
                            rhs=w1_sb[:, k * P:(k + 1) * P],
                            start=(k == 0), stop=(k == nk - 1),
                        )
                else:
                    q = q_of_t(t)
                    off = (t - Q_TSTART[q]) * P
                    w = w2_sb if l == 1 else w3_sb
                    nc.tensor.matmul(
                        out=ps[:r, :], lhsT=current_aT[l][q][:, off:off + r], rhs=w[:],
                        start=True, stop=True,
                    )
                hs = hsp.tile([P, P], dt.bfloat16, tag="hs")
                nc.vector.tensor_copy(hs[:r, :], ps[:r, :])
                nc.sync.dma_start(bounces[l][c0:c0 + r, :], hs[:r, :])

            def emit_allgathers(l):
                for q in range(4):
                    r0 = Q_RSTART[q]
                    nc.gpsimd.collective_compute(
                        "AllGather", mybir.AluOpType.bypass,
                        replica_groups=[list(range(N_CORES))],
                        ins=[bounces[l][r0:r0 + Q_ROWS[q], :]],
                        outs=[tbls[l][q][:]],
                    )

            # ---- dense layer 0: staged x stripes, two node-halves ----
            xhalves = []
            with tc.tile_pool(name="xs", bufs=5) as xsp:
                for h in range(2):
                    hb = 0 if h == 0 else 49 * P
                    hc = 49 * P if h == 0 else NODES_PER_CORE - 49 * P
                    ks = []
                    for k in range(F0 // P):
                        xk = xsp.tile([P, 49 * P], dt.bfloat16, tag="xh")
                        nc.sync.dma_start(
                            xk[:, :hc], xT_io[k * P:(k + 1) * P, hb:hb + hc]
                        )
                        ks.append(xk)
                    xhalves.append(ks)
                for t in range(NTILES):
                    dense_tile(0, t)
                emit_allgathers(0)

            def agg_layer(l, idxp, msgp, selp, workp):
                ncols = NCOLS_MM[l]
                if l < 2:
                    current_aT[l + 1] = [
                        aTp.tile([P, Q_ROWS[q]], dt.bfloat16, tag=f"aq{q}",
                                 name=f"a{l + 1}q{q}")
                        for q in range(4)
                    ]
                for w, wtiles in enumerate(waves):
                    # gathers (one call per chunk, queue = chunk)
                    msg_tiles = {}
                    for ch in range(NCHUNK):
                        nb = int(sum(blocks[t, ch] for t in wtiles))
                        s0 = chunk_start[ch][wtiles[0]]
                        S = nb * P
                        iw = idxp.tile([P, max_nb * P // 16], dt.int16, tag="idx")
                        nc.sync.dma_start(iw[:, :S // 16], idx_ios[ch][:, s0 // 16:(s0 + S) // 16])
                        mt = msgp.tile([P, max_nb, FW], dt.bfloat16, tag="msg")
                        for b0 in range(0, nb, MAX_CALL_BLOCKS):
                            b1 = min(b0 + MAX_CALL_BLOCKS, nb)
                            Ssub = (b1 - b0) * P
                            nc.gpsimd.dma_gather(
                                out_ap=mt[:, b0:b1, :],
                                in_ap=tbls[l][ch][:],
                                idxs_ap=iw[:, b0 * P // 16:b1 * P // 16],
                                num_idxs=Ssub, num_idxs_reg=Ssub,
                                elem_size=FW, elem_step=FW,
                                single_packet=False,
                                queue_num=ch % NQUEUES,
                            )
                        msg_tiles[ch] = mt

                    # selection matrices, batches of 8 blocks in didx col order
                    # sel blocks: [slot, dst, block] layout so every operand's
                    # last dim is stride-1 (enables the DVE 2x mode)
                    gw0 = g_start[(wtiles[0], 0)]
                    gw1 = gw0 + int(sum(blocks[t, ch] for t in wtiles for ch in range(NCHUNK)))
                    sel_tiles = {}
                    for q0 in range(gw0, gw1, BB):
                        q1 = min(q0 + BB, gw1)
                        st = selp.tile([P, P, BB], dt.bfloat16, tag="sel")
                        nc.vector.tensor_tensor(
                            out=st[:, :, :q1 - q0],
                            in0=didx_sb[:, None, q0:q1].to_broadcast([P, P, q1 - q0]),
                            in1=iotaR[:, :, :q1 - q0],
                            op=mybir.AluOpType.is_equal,
                        )
                        for q in range(q0, q1):
                            sel_tiles[q] = (st, q - q0)

                    # per-tile matmul accumulation + post + (dense l+1)
                    for ti, t in enumerate(wtiles):
                        r = _rows(t)
                        c0 = t * P
                        ps = psap.tile([P, P], dt.float32, space="PSUM", tag="pa")
                        mms = []
                        for ch in range(NCHUNK):
                            coff = int(sum(blocks[tt, ch] for tt in wtiles[:ti]))
                            for b in range(int(blocks[t, ch])):
                                mms.append((g_start[(t, ch)] + b, ch, coff + b))
                        for i, (q, ch, col) in enumerate(mms):
                            st, j = sel_tiles[q]
                            nc.tensor.matmul(
                                out=ps[:r, :ncols], lhsT=st[:, :r, j],
                                rhs=msg_tiles[ch][:, col, :ncols],
                                start=(i == 0),
                                stop=(i == len(mms) - 1),
                            )
                        if l < 2:
                            outt = workp.tile([P, P], dt.bfloat16, tag="outb")
                            nc.scalar.activation(
                                outt[:r, :], ps[:r, :],
                                mybir.ActivationFunctionType.Relu,
                                scale=sc_in[:r, t:t + 1],
                            )
                            pst = pstp.tile([P, P], dt.bfloat16, space="PSUM", tag="pt")
                            nc.tensor.transpose(
                                out=pst[:, :r], in_=outt[:r, :], identity=identb[:r, :r]
                            )
                            q = q_of_t(t)
                            off = (t - Q_TSTART[q]) * P
                            nc.scalar.activation(
                                current_aT[l + 1][q][:, off:off + r], pst[:, :r],
                                mybir.ActivationFunctionType.Copy,
                            )
                            dense_tile(l + 1, t)
                        else:
                            outt = workp.tile([P, 64], dt.float32, tag="outf")
                            nc.scalar.activation(
                                outt[:r, :], ps[:r, :64],
                                mybir.ActivationFunctionType.Relu,
                                scale=sc_out[:r, t:t + 1],
                            )
                            nc.sync.dma_start(out_io[c0:c0 + r, :], outt[:r, :])
                if l < 2:
                    emit_allgathers(l + 1)

            with (
                tc.tile_pool(name="idxw", bufs=10) as idxp,
                tc.tile_pool(name="msgs", bufs=8) as msgp,
                tc.tile_pool(name="sel", bufs=6) as selp,
                tc.tile_pool(name="work", bufs=6) as workp,
            ):
                agg_layer(0, idxp, msgp, selp, workp)
                agg_layer(1, idxp, msgp, selp, workp)
                agg_layer(2, idxp, msgp, selp, workp)

    nc.compile()
    return nc


def _pack_inputs(x, dinv, W1, W2, W3, idx16, didx):
    iota_rep = np.tile(np.arange(P, dtype=np.float32)[None, :], (P, 1)).astype(BF)
    iotaR = np.tile(
        np.repeat(np.arange(P, dtype=np.float32), BB)[None, :], (P, 1)
    ).astype(BF)  # [p, j*BB + g] = j
    w1sb = np.zeros((P, F0), np.float32)
    for k in range(F0 // P):
        w1sb[:, k * P:(k + 1) * P] = W1[k * P:(k + 1) * P, :]
    w2pad = np.zeros((P, P), np.float32)
    w2pad[:, :64] = W2
    w3pad = np.zeros((P, P), np.float32)
    w3pad[:64, :32] = W3

    in_maps = []
    for c in range(N_CORES):
        lo = c * NODES_PER_CORE
        dv = dinv[lo:lo + NODES_PER_CORE]
        xs = x[lo:lo + NODES_PER_CORE].astype(np.float32) * dv[:, None]
        sc_in = np.ones((P, NTILES), np.float32)   # dinv^2 per tile col
        sc_out = np.ones((P, NTILES), np.float32)  # dinv per tile col
        for t in range(NTILES):
            r = _rows(t)
            sc_in[:r, t] = dv[t * P:t * P + r] ** 2
            sc_out[:r, t] = dv[t * P:t * P + r]
        in_maps.append({
            "xT": np.ascontiguousarray(xs.T).astype(BF),
            "scIn": sc_in,
            "scOut": sc_out,
            "W1sb": w1sb.astype(BF),
            "W2pad": w2pad.astype(BF),
            "W3pad": w3pad.astype(BF),
            "iota": iota_rep,
            "iotaR": iotaR,
            **{f"idx{ch}": idx16[ch][c] for ch in range(NCHUNK)},
            "didx": didx[c].astype(BF),
        })
    return in_maps


_TRACE = [False]          # set by test harness to request a profiled run
_LAST_RESULT = [None]     # BassKernelResults of the last run (for profiling)


def kernel(x, edge_index, batch, W1, b1, W2, b2, W3, b3, Wfc, bfc):
    x = np.asarray(x)
    edge_index = np.asarray(edge_index)
    batch = np.asarray(batch)
    W1, b1 = np.asarray(W1), np.asarray(b1)
    W2, b2 = np.asarray(W2), np.asarray(b2)
    W3, b3 = np.asarray(W3), np.asarray(b3)
    Wfc, bfc = np.asarray(Wfc), np.asarray(bfc)
    has_bias = bool(np.abs(b1).max() or np.abs(b2).max() or np.abs(b3).max())
    assert not has_bias, "nonzero conv biases not wired up in v2"

    dinv, idx16, didx, meta = _host_prep(edge_index.astype(np.int64))
    nc = _build_program(meta, has_bias)
    in_maps = _pack_inputs(x, dinv, W1, W2, W3, idx16, didx)
    res = run_bass_kernel_spmd(
        nc, in_maps, core_ids=list(range(N_CORES)), trace=_TRACE[0]
    )
    _LAST_RESULT[0] = res

    h3 = np.concatenate([res.results[c]["out3"][:, :32] for c in range(N_CORES)], axis=0)

    # host epilogue: segment max pool + FC + log_softmax (float64 for stability)
    pooled = np.full((N_GRAPHS, 32), -np.inf, np.float64)
    bnd = np.searchsorted(batch, np.arange(N_GRAPHS + 1))
    for g in range(N_GRAPHS):
        if bnd[g + 1] > bnd[g]:
            pooled[g] = h3[bnd[g]:bnd[g + 1]].max(axis=0)
    logits = pooled @ Wfc.astype(np.float64) + bfc.astype(np.float64)
    m = logits.max(axis=1, keepdims=True)
    lse = m + np.log(np.exp(logits - m).sum(axis=1, keepdims=True))
    return (logits - lse).astype(np.float32)


# revision 32
# speedup vs baseline: 1.4216x; 1.1171x over previous
"""GCN (3x GCNConv + global max pool + FC + log_softmax) on 8 Trainium2 NeuronCores.

v2 — pipelined rewrite of the working baseline:
  - 1D node partition: core c owns rows [12500c, 12500(c+1)).
  - dinv folded into inputs: x_pre = dinv*x on host; each conv's post is a single
    Act-engine relu with per-partition scale (dinv^2 for inner convs, dinv for the
    last) since relu(c*y) = c*relu(y) for c>0 and D(AW) = (DA)W.
  - dense(l): hs = a_pre @ W_l per local tile (PE, bf16) -> hs tile -> bounce DRAM.
    Quarter-granular AllGather (4 per layer, Shared outputs) so collectives start
    as soon as a quarter of the dense outputs lands and overlap with aggregation.
  - Aggregation: per-edge gather of table rows via gpsimd dma_gather (int16 idxs,
    4 src chunks = the AllGather quarters), then segment-sum via PE matmuls with
    per-128-edge one-hot selection matrices built on DVE, accumulated in PSUM per
    128-dst tile.  Gather idxs are src-sorted within groups; padding slots are -1
    (skipped by the DMA ucode).
  - dense(l+1) tiles are emitted immediately after each agg(l) tile so layers
    pipeline; AllGather triggers sit between layers' gather streams on gpsimd.
  - Pooling (segment max over graphs), tiny FC and log_softmax run on host.
"""

import sys

sys.path.insert(0, "/opt/trn_rl_repo")

import numpy as np
import ml_dtypes

import concourse.bass as bass
import concourse.bacc as bacc
import concourse.tile as tile
from concourse import mybir
from concourse.bass_utils import run_bass_kernel_spmd
from concourse.masks import make_identity

P = 128
N_NODES = 100000
N_EDGES = 1600000
N_GRAPHS = 64
N_CORES = 8
NODES_PER_CORE = N_NODES // N_CORES          # 12500
NTILES = (NODES_PER_CORE + P - 1) // P       # 98 (last tile 84 rows)
LAST_ROWS = NODES_PER_CORE - (NTILES - 1) * P  # 84
NCHUNK = 4
QT = [25, 25, 25, 23]                        # quarter tile counts
Q_TSTART = [0, 25, 50, 75]
Q_ROWS = [3200, 3200, 3200, 2900]            # per-core rows per quarter
Q_RSTART = [0, 3200, 6400, 9600]
CH_ROWS = [r * N_CORES for r in Q_ROWS]      # chunk rows: 25600,25600,25600,23200
WT = 4                                       # dst tiles per wave
F0 = 512
FW = 128                                     # table width (padded)
NCOLS_MM = [128, 64, 32]                     # agg matmul N per layer
MAX_CALL_BLOCKS = 47                         # ~6K idxs per dma_gather call
BB = 16                                      # sel blocks built per DVE instruction
NQUEUES = 4
DMA_SCRATCH = 16384
dt = mybir.dt
BF = ml_dtypes.bfloat16


def _rows(t):
    return LAST_ROWS if t == NTILES - 1 else P


def _host_prep(edge_index):
    """Build the shared (cross-core) aggregation schedule + per-core index data."""
    src = np.concatenate([edge_index[0], np.arange(N_NODES, dtype=np.int64)])
    dst = np.concatenate([edge_index[1], np.arange(N_NODES, dtype=np.int64)])
    deg = np.bincount(dst, minlength=N_NODES).astype(np.float32)
    dinv = (1.0 / np.sqrt(deg)).astype(np.float32)

    waves = [list(range(w, min(w + WT, NTILES))) for w in range(0, NTILES, WT)]

    # src -> (chunk, row-in-chunk): chunk = quarter of the owner's tile range
    s_core = src // NODES_PER_CORE
    s_loc = src % NODES_PER_CORE
    s_t = s_loc // P
    qs = np.searchsorted(np.array([25, 50, 75]), s_t, side="right")  # quarter id
    q_rows = np.array(Q_ROWS)
    q_rstart = np.array(Q_RSTART)
    s_row = s_core * q_rows[qs] + (s_loc - q_rstart[qs])

    per_core = []
    cnts = np.zeros((N_CORES, NTILES, NCHUNK), np.int64)
    d_core = dst // NODES_PER_CORE
    for c in range(N_CORES):
        m = d_core == c
        srow, sq = s_row[m], qs[m]
        dl = dst[m] - c * NODES_PER_CORE
        t = dl // P
        key = t * NCHUNK + sq
        o = np.lexsort((srow, key))          # group by (t,chunk), src-sorted within
        srow, dl, key = srow[o], dl[o], key[o]
        cnt = np.bincount(key, minlength=NTILES * NCHUNK).reshape(NTILES, NCHUNK)
        cnts[c] = cnt
        per_core.append((srow, dl, cnt))

    blocks = np.maximum((cnts.max(axis=0) + P - 1) // P, 1)  # [NTILES, NCHUNK]

    S_ch = [int(blocks[:, ch].sum() * P) for ch in range(NCHUNK)]
    NB_total = int(blocks.sum())

    # idx slot order per chunk: (wave, tile, block); didx col order: (wave, tile, chunk, block)
    chunk_start = [dict() for _ in range(NCHUNK)]
    for ch in range(NCHUNK):
        pos = 0
        for wtiles in waves:
            for t in wtiles:
                chunk_start[ch][t] = pos
                pos += int(blocks[t, ch]) * P
    g_start = {}
    g = 0
    for wtiles in waves:
        for t in wtiles:
            for ch in range(NCHUNK):
                g_start[(t, ch)] = g
                g += int(blocks[t, ch])
    assert g == NB_total

    idx16 = [np.zeros((N_CORES, P, S_ch[ch] // 16), np.int16) for ch in range(NCHUNK)]
    didx = np.full((N_CORES, P, NB_total), -1.0, np.float32)

    for c in range(N_CORES):
        srow, dl, cnt = per_core[c]
        ends = np.cumsum(cnt.reshape(-1))
        starts = ends - cnt.reshape(-1)
        idx_slots = [np.zeros(S_ch[ch], np.int16) for ch in range(NCHUNK)]
        didx_slots = np.full(NB_total * P, -1.0, np.float32)
        for t in range(NTILES):
            for ch in range(NCHUNK):
                k = t * NCHUNK + ch
                n = cnt[t, ch]
                if n == 0:
                    continue
                e0, e1 = starts[k], ends[k]
                cs = chunk_start[ch][t]
                idx_slots[ch][cs:cs + n] = srow[e0:e1].astype(np.int16)
                gs = g_start[(t, ch)] * P
                didx_slots[gs:gs + n] = (dl[e0:e1] - t * P).astype(np.float32)
        for ch in range(NCHUNK):
            w16 = idx_slots[ch].reshape(-1, 16).T  # [16, S/16]
            idx16[ch][c] = np.tile(w16, (8, 1))
        didx[c] = didx_slots.reshape(-1, P).T

    max_nb = max(
        int(sum(blocks[t, ch] for t in wtiles))
        for wtiles in waves for ch in range(NCHUNK)
    )
    meta = {
        "waves": waves,
        "blocks": blocks,
        "S_ch": S_ch,
        "NB_total": NB_total,
        "chunk_start": chunk_start,
        "g_start": g_start,
        "max_nb": max_nb,
    }
    return dinv, idx16, didx, meta


def _build_program(meta, has_bias):
    waves = meta["waves"]
    blocks = meta["blocks"]
    S_ch = meta["S_ch"]
    NB_total = meta["NB_total"]
    chunk_start = meta["chunk_start"]
    g_start = meta["g_start"]
    max_nb = meta["max_nb"]

    nc = bacc.Bacc(
        "TRN2", target_bir_lowering=False, debug=False, num_devices=N_CORES,
        num_swdge_queues=NQUEUES, dynamic_dma_scratch_size=DMA_SCRATCH,
    )

    xT_io = nc.dram_tensor("xT", [F0, NODES_PER_CORE], dt.bfloat16, kind="ExternalInput").ap()
    sc_in_io = nc.dram_tensor("scIn", [P, NTILES], dt.float32, kind="ExternalInput").ap()   # dinv^2
    sc_out_io = nc.dram_tensor("scOut", [P, NTILES], dt.float32, kind="ExternalInput").ap() # dinv
    w1_io = nc.dram_tensor("W1sb", [P, F0], dt.bfloat16, kind="ExternalInput").ap()
    w2_io = nc.dram_tensor("W2pad", [P, P], dt.bfloat16, kind="ExternalInput").ap()
    w3_io = nc.dram_tensor("W3pad", [P, P], dt.bfloat16, kind="ExternalInput").ap()
    iota_io = nc.dram_tensor("iota", [P, P], dt.bfloat16, kind="ExternalInput").ap()
    iotaR_io = nc.dram_tensor("iotaR", [P, P * BB], dt.bfloat16, kind="ExternalInput").ap()
    idx_ios = [
        nc.dram_tensor(f"idx{ch}", [P, S_ch[ch] // 16], dt.int16, kind="ExternalInput").ap()
        for ch in range(NCHUNK)
    ]
    didx_io = nc.dram_tensor("didx", [P, NB_total], dt.bfloat16, kind="ExternalInput").ap()
    out_io = nc.dram_tensor("out3", [NODES_PER_CORE, 32], dt.float32, kind="ExternalOutput").ap()

    with tile.TileContext(nc) as tc:
        with (
            tc.tile_pool(name="const", bufs=1) as constp,
            tc.tile_pool(name="aT", bufs=1) as aTp,
            tc.tile_pool(name="hs", bufs=8) as hsp,
            tc.tile_pool(name="psum_d", bufs=2, space="PSUM") as psdp,
            tc.tile_pool(name="psum_a", bufs=4, space="PSUM") as psap,
            tc.tile_pool(name="psum_t", bufs=2, space="PSUM") as pstp,
            tc.tile_pool(name="dram", bufs=1, space="DRAM") as dramp,
        ):
            # ---- constants ----
            iota_t = constp.tile([P, P], dt.bfloat16)
            nc.sync.dma_start(iota_t[:], iota_io[:])
            sc_in = constp.tile([P, NTILES], dt.float32)
            nc.sync.dma_start(sc_in[:], sc_in_io[:])
            sc_out = constp.tile([P, NTILES], dt.float32)
            nc.sync.dma_start(sc_out[:], sc_out_io[:])
            w1_sb = constp.tile([P, F0], dt.bfloat16)
            nc.sync.dma_start(w1_sb[:], w1_io[:])
            w2_sb = constp.tile([P, P], dt.bfloat16)
            nc.sync.dma_start(w2_sb[:], w2_io[:])
            w3_sb = constp.tile([P, P], dt.bfloat16)
            nc.sync.dma_start(w3_sb[:], w3_io[:])
            didx_sb = constp.tile([P, NB_total], dt.bfloat16)
            nc.sync.dma_start(didx_sb[:], didx_io[:])
            iotaR = constp.tile([P, P, BB], dt.bfloat16)   # [p, j, g] = j
            nc.sync.dma_start(iotaR[:], iotaR_io[:])
            identb = constp.tile([P, P], dt.bfloat16)
            make_identity(nc, identb[:])

            # a_preT tiles (one set, reallocated per layer): [128 feats, qrows]
            current_aT = {}

            bounces = [
                dramp.tile([NODES_PER_CORE, FW], dt.bfloat16, tag=f"bnc{l}", name=f"bnc{l}")
                for l in range(3)
            ]
            tbls = [
                [dramp.tile([CH_ROWS[q], FW], dt.bfloat16, tag=f"tbl{l}q{q}",
                            name=f"tbl{l}q{q}", addr_space="Shared") for q in range(4)]
                for l in range(3)
            ]

            def q_of_t(t):
                return min(t // 25, 3)

            def dense_tile(l, t):
                """hs tile = a_pre @ W  (psum) -> bf16 hs tile -> bounce DRAM."""
                r = _rows(t)
                c0 = t * P
                ps = psdp.tile([P, P], dt.float32, space="PSUM", tag="pd")
                if l == 0:
                    h = 0 if t < 49 else 1
                    hb = 0 if t < 49 else 49 * P
                    nk = F0 // P
                    for k in range(nk):
                        xk = xhalves[h][k]
                        nc.tensor.matmul(
                            out=ps[:r, :], lhsT=xk[:, c0 - hb:c0 - hb + r],
                            rhs=w1_sb[:, k * P:(k + 1) * P],
                            start=(k == 0), stop=(k == nk - 1),
                        )
                else:
                    q = q_of_t(t)
                    off = (t - Q_TSTART[q]) * P
                    w = w2_sb if l == 1 else w3_sb
                    nc.tensor.matmul(
                        out=ps[:r, :], lhsT=current_aT[l][q][:, off:off + r], rhs=w[:],
                        start=True, stop=True,
                    )
                hs = hsp.tile([P, P], dt.bfloat16, tag="hs")
                nc.vector.tensor_copy(hs[:r, :], ps[:r, :])
                nc.sync.dma_start(bounces[l][c0:c0 + r, :], hs[:r, :])

            def emit_allgathers(l):
                for q in range(4):
                    r0 = Q_RSTART[q]
                    nc.gpsimd.collective_compute(
                        "AllGather", mybir.AluOpType.bypass,
                        replica_groups=[list(range(N_CORES))],
                        ins=[bounces[l][r0:r0 + Q_ROWS[q], :]],
                        outs=[tbls[l][q][:]],
                    )

            # ---- dense layer 0: staged x stripes, two node-halves ----
            xhalves = []
            with tc.tile_pool(name="xs", bufs=5) as xsp:
                for h in range(2):
                    hb = 0 if h == 0 else 49 * P
                    hc = 49 * P if h == 0 else NODES_PER_CORE - 49 * P
                    ks = []
                    for k in range(F0 // P):
                        xk = xsp.tile([P, 49 * P], dt.bfloat16, tag="xh")
                        nc.sync.dma_start(
                            xk[:, :hc], xT_io[k * P:(k + 1) * P, hb:hb + hc]
                        )
                        ks.append(xk)
                    xhalves.append(ks)
                for t in range(NTILES):
                    dense_tile(0, t)
                emit_allgathers(0)

            def agg_layer(l, idxp, msgp, selp, workp):
                ncols = NCOLS_MM[l]
                if l < 2:
                    current_aT[l + 1] = [
                        aTp.tile([P, Q_ROWS[q]], dt.bfloat16, tag=f"aq{q}",
                                 name=f"a{l + 1}q{q}")
                        for q in range(4)
                    ]
                for w, wtiles in enumerate(waves):
                    # gathers (one call per chunk, queue = chunk)
                    msg_tiles = {}
                    for ch in range(NCHUNK):
                        nb = int(sum(blocks[t, ch] for t in wtiles))
                        s0 = chunk_start[ch][wtiles[0]]
                        S = nb * P
                        iw = idxp.tile([P, max_nb * P // 16], dt.int16, tag="idx")
                        nc.sync.dma_start(iw[:, :S // 16], idx_ios[ch][:, s0 // 16:(s0 + S) // 16])
                        mt = msgp.tile([P, max_nb, FW], dt.bfloat16, tag="msg")
                        for b0 in range(0, nb, MAX_CALL_BLOCKS):
                            b1 = min(b0 + MAX_CALL_BLOCKS, nb)
                            Ssub = (b1 - b0) * P
                            nc.gpsimd.dma_gather(
                                out_ap=mt[:, b0:b1, :],
                                in_ap=tbls[l][ch][:],
                                idxs_ap=iw[:, b0 * P // 16:b1 * P // 16],
                                num_idxs=Ssub, num_idxs_reg=Ssub,
                                elem_size=FW, elem_step=FW,
                                single_packet=False,
                                queue_num=ch % NQUEUES,
                            )
                        msg_tiles[ch] = mt

                    # selection matrices, batches of 8 blocks in didx col order
                    # sel blocks: [slot, dst, block] layout so every operand's
                    # last dim is stride-1 (enables the DVE 2x mode)
                    gw0 = g_start[(wtiles[0], 0)]
                    gw1 = gw0 + int(sum(blocks[t, ch] for t in wtiles for ch in range(NCHUNK)))
                    sel_tiles = {}
                    for q0 in range(gw0, gw1, BB):
                        q1 = min(q0 + BB, gw1)
                        st = selp.tile([P, P, BB], dt.bfloat16, tag="sel")
                        nc.vector.tensor_tensor(
                            out=st[:, :, :q1 - q0],
                            in0=didx_sb[:, None, q0:q1].to_broadcast([P, P, q1 - q0]),
                            in1=iotaR[:, :, :q1 - q0],
                            op=mybir.AluOpType.is_equal,
                        )
                        for q in range(q0, q1):
                            sel_tiles[q] = (st, q - q0)

                    # per-tile matmul accumulation + post + (dense l+1)
                    for ti, t in enumerate(wtiles):
                        r = _rows(t)
                        c0 = t * P
                        ps = psap.tile([P, P], dt.float32, space="PSUM", tag="pa")
                        mms = []
                        for ch in range(NCHUNK):
                            coff = int(sum(blocks[tt, ch] for tt in wtiles[:ti]))
                            for b in range(int(blocks[t, ch])):
                                mms.append((g_start[(t, ch)] + b, ch, coff + b))
                        for i, (q, ch, col) in enumerate(mms):
                            st, j = sel_tiles[q]
                            nc.tensor.matmul(
                                out=ps[:r, :ncols], lhsT=st[:, :r, j],
                                rhs=msg_tiles[ch][:, col, :ncols],
                                start=(i == 0),
                                stop=(i == len(mms) - 1),
                            )
                        if l < 2:
                            outt = workp.tile([P, P], dt.bfloat16, tag="outb")
                            nc.scalar.activation(
                                outt[:r, :], ps[:r, :],
                                mybir.ActivationFunctionType.Relu,
                                scale=sc_in[:r, t:t + 1],
                            )
                            pst = pstp.tile([P, P], dt.bfloat16, space="PSUM", tag="pt")
                            nc.tensor.transpose(
                                out=pst[:, :r], in_=outt[:r, :], identity=identb[:r, :r]
                            )
                            q = q_of_t(t)
                            off = (t - Q_TSTART[q]) * P
                            nc.scalar.activation(
                                current_aT[l + 1][q][:, off:off + r], pst[:, :r],
                                mybir.ActivationFunctionType.Copy,
                            )
                            dense_tile(l + 1, t)
                        else:
                            outt = workp.tile([P, 32], dt.float32, tag="outf")
                            nc.scalar.activation(
                                outt[:r, :], ps[:r, :32],
                                mybir.ActivationFunctionType.Relu,
                                scale=sc_out[:r, t:t + 1],
                            )
                            nc.sync.dma_start(out_io[c0:c0 + r, :], outt[:r, :])
                if l < 2:
                    emit_allgathers(l + 1)

            with (
                tc.tile_pool(name="idxw", bufs=20) as idxp,
                tc.tile_pool(name="msgs", bufs=16) as msgp,
                tc.tile_pool(name="sel", bufs=8) as selp,
                tc.tile_pool(name="work", bufs=6) as workp,
            ):
                agg_layer(0, idxp, msgp, selp, workp)
                agg_layer(1, idxp, msgp, selp, workp)
                agg_layer(2, idxp, msgp, selp, workp)

    nc.compile()
    return nc


def _pack_inputs(x, dinv, W1, W2, W3, idx16, didx):
    iota_rep = np.tile(np.arange(P, dtype=np.float32)[None, :], (P, 1)).astype(BF)
    iotaR = np.tile(
        np.repeat(np.arange(P, dtype=np.float32), BB)[None, :], (P, 1)
    ).astype(BF)  # [p, j*BB + g] = j
    w1sb = np.zeros((P, F0), np.float32)
    for k in range(F0 // P):
        w1sb[:, k * P:(k + 1) * P] = W1[k * P:(k + 1) * P, :]
    w2pad = np.zeros((P, P), np.float32)
    w2pad[:, :64] = W2
    w3pad = np.zeros((P, P), np.float32)
    w3pad[:64, :32] = W3

    in_maps = []
    for c in range(N_CORES):
        lo = c * NODES_PER_CORE
        dv = dinv[lo:lo + NODES_PER_CORE]
        xs = x[lo:lo + NODES_PER_CORE].astype(np.float32) * dv[:, None]
        sc_in = np.ones((P, NTILES), np.float32)   # dinv^2 per tile col
        sc_out = np.ones((P, NTILES), np.float32)  # dinv per tile col
        for t in range(NTILES):
            r = _rows(t)
            sc_in[:r, t] = dv[t * P:t * P + r] ** 2
            sc_out[:r, t] = dv[t * P:t * P + r]
        in_maps.append({
            "xT": np.ascontiguousarray(xs.T).astype(BF),
            "scIn": sc_in,
            "scOut": sc_out,
            "W1sb": w1sb.astype(BF),
            "W2pad": w2pad.astype(BF),
            "W3pad": w3pad.astype(BF),
            "iota": iota_rep,
            "iotaR": iotaR,
            **{f"idx{ch}": idx16[ch][c] for ch in range(NCHUNK)},
            "didx": didx[c].astype(BF),
        })
    return in_maps


_TRACE = [False]          # set by test harness to request a profiled run
_LAST_RESULT = [None]     # BassKernelResults of the last run (for profiling)


def kernel(x, edge_index, batch, W1, b1, W2, b2, W3, b3, Wfc, bfc):
    x = np.asarray(x)
    edge_index = np.asarray(edge_index)
    batch = np.asarray(batch)
    W1, b1 = np.asarray(W1), np.asarray(b1)
    W2, b2 = np.asarray(W2), np.asarray(b2)
    W3, b3 = np.asarray(W3), np.asarray(b3)
    Wfc, bfc = np.asarray(Wfc), np.asarray(bfc)
    has_bias = bool(np.abs(b1).max() or np.abs(b2).max() or np.abs(b3).max())
    assert not has_bias, "nonzero conv biases not wired up in v2"

    dinv, idx16, didx, meta = _host_prep(edge_index.astype(np.int64))
    nc = _build_program(meta, has_bias)
    in_maps = _pack_inputs(x, dinv, W1, W2, W3, idx16, didx)
    res = run_bass_kernel_spmd(
        nc, in_maps, core_ids=list(range(N_CORES)), trace=_TRACE[0]
    )
    _LAST_RESULT[0] = res

    h3 = np.concatenate([res.results[c]["out3"][:, :32] for c in range(N_CORES)], axis=0)

    # host epilogue: segment max pool + FC + log_softmax (float64 for stability)
    pooled = np.full((N_GRAPHS, 32), -np.inf, np.float64)
    bnd = np.searchsorted(batch, np.arange(N_GRAPHS + 1))
    for g in range(N_GRAPHS):
        if bnd[g + 1] > bnd[g]:
            pooled[g] = h3[bnd[g]:bnd[g + 1]].max(axis=0)
    logits = pooled @ Wfc.astype(np.float64) + bfc.astype(np.float64)
    m = logits.max(axis=1, keepdims=True)
    lse = m + np.log(np.exp(logits - m).sum(axis=1, keepdims=True))
    return (logits - lse).astype(np.float32)


# revision 37
# speedup vs baseline: 1.9403x; 1.3648x over previous
"""GCN (3x GCNConv + global max pool + FC + log_softmax) on 8 Trainium2 NeuronCores.

v2 — pipelined rewrite of the working baseline:
  - 1D node partition: core c owns rows [12500c, 12500(c+1)).
  - dinv folded into inputs: x_pre = dinv*x on host; each conv's post is a single
    Act-engine relu with per-partition scale (dinv^2 for inner convs, dinv for the
    last) since relu(c*y) = c*relu(y) for c>0 and D(AW) = (DA)W.
  - dense(l): hs = a_pre @ W_l per local tile (PE, bf16) -> hs tile -> bounce DRAM.
    Quarter-granular AllGather (4 per layer, Shared outputs) so collectives start
    as soon as a quarter of the dense outputs lands and overlap with aggregation.
  - Aggregation: per-edge gather of table rows via gpsimd dma_gather (int16 idxs,
    4 src chunks = the AllGather quarters), then segment-sum via PE matmuls with
    per-128-edge one-hot selection matrices built on DVE, accumulated in PSUM per
    128-dst tile.  Gather idxs are src-sorted within groups; padding slots are -1
    (skipped by the DMA ucode).
  - dense(l+1) tiles are emitted immediately after each agg(l) tile so layers
    pipeline; AllGather triggers sit between layers' gather streams on gpsimd.
  - Pooling (segment max over graphs), tiny FC and log_softmax run on host.
"""

import sys

sys.path.insert(0, "/opt/trn_rl_repo")

import numpy as np
import ml_dtypes

import concourse.bass as bass
import concourse.bacc as bacc
import concourse.tile as tile
from concourse import mybir
from concourse.bass_utils import run_bass_kernel_spmd
from concourse.masks import make_identity

P = 128
N_NODES = 100000
N_EDGES = 1600000
N_GRAPHS = 64
N_CORES = 8
NODES_PER_CORE = N_NODES // N_CORES          # 12500
NTILES = (NODES_PER_CORE + P - 1) // P       # 98 (last tile 84 rows)
LAST_ROWS = NODES_PER_CORE - (NTILES - 1) * P  # 84
NCHUNK = 4
QT = [25, 25, 25, 23]                        # quarter tile counts
Q_TSTART = [0, 25, 50, 75]
Q_ROWS = [3200, 3200, 3200, 2900]            # per-core rows per quarter
Q_RSTART = [0, 3200, 6400, 9600]
CH_ROWS = [r * N_CORES for r in Q_ROWS]      # chunk rows: 25600,25600,25600,23200
WT = 4                                       # dst tiles per wave
F0 = 512
FW = 128                                     # table width (padded)
NCOLS_MM = [128, 64, 32]                     # agg matmul N per layer
MAX_CALL_BLOCKS = 47                         # ~6K idxs per dma_gather call
BB = 16                                      # sel blocks built per DVE instruction
NQUEUES = 4
DMA_SCRATCH = 16384
dt = mybir.dt
BF = ml_dtypes.bfloat16


def _rows(t):
    return LAST_ROWS if t == NTILES - 1 else P


def _host_prep(edge_index):
    """Build the shared (cross-core) aggregation schedule + per-core index data."""
    src = np.concatenate([edge_index[0], np.arange(N_NODES, dtype=np.int64)])
    dst = np.concatenate([edge_index[1], np.arange(N_NODES, dtype=np.int64)])
    deg = np.bincount(dst, minlength=N_NODES).astype(np.float32)
    dinv = (1.0 / np.sqrt(deg)).astype(np.float32)

    waves = [list(range(w, min(w + WT, NTILES))) for w in range(0, NTILES, WT)]

    # src -> (chunk, row-in-chunk): chunk = quarter of the owner's tile range
    s_core = src // NODES_PER_CORE
    s_loc = src % NODES_PER_CORE
    s_t = s_loc // P
    qs = np.searchsorted(np.array([25, 50, 75]), s_t, side="right")  # quarter id
    q_rows = np.array(Q_ROWS)
    q_rstart = np.array(Q_RSTART)
    s_row = s_core * q_rows[qs] + (s_loc - q_rstart[qs])

    per_core = []
    cnts = np.zeros((N_CORES, NTILES, NCHUNK), np.int64)
    d_core = dst // NODES_PER_CORE
    for c in range(N_CORES):
        m = d_core == c
        srow, sq = s_row[m], qs[m]
        dl = dst[m] - c * NODES_PER_CORE
        t = dl // P
        key = t * NCHUNK + sq
        o = np.lexsort((srow, key))          # group by (t,chunk), src-sorted within
        srow, dl, key = srow[o], dl[o], key[o]
        cnt = np.bincount(key, minlength=NTILES * NCHUNK).reshape(NTILES, NCHUNK)
        cnts[c] = cnt
        per_core.append((srow, dl, cnt))

    NW = len(waves)
    # contiguous per-(wave,chunk) slot packing: per-core tile segments are
    # concatenated without per-tile 128-padding; only the (w,ch) region is
    # padded to a block multiple (shared across cores via max).
    wcnt = np.zeros((N_CORES, NW, NCHUNK), np.int64)
    for c in range(N_CORES):
        for w, wtiles in enumerate(waves):
            for ch in range(NCHUNK):
                wcnt[c, w, ch] = cnts[c, wtiles, ch].sum()
    blocks2 = np.maximum((wcnt.max(axis=0) + P - 1) // P, 1)  # [NW, NCHUNK]

    chunk_start = [dict() for _ in range(NCHUNK)]   # [ch][w] -> slot offset
    for ch in range(NCHUNK):
        pos = 0
        for w in range(NW):
            chunk_start[ch][w] = pos
            pos += int(blocks2[w, ch]) * P
    S_ch = [int(blocks2[:, ch].sum() * P) for ch in range(NCHUNK)]

    # union over cores of (tile, block) pairs per (w, ch)
    entry_set = [set() for _ in range(NW)]
    for c in range(N_CORES):
        for w, wtiles in enumerate(waves):
            for ch in range(NCHUNK):
                s = 0
                for t in wtiles:
                    n = int(cnts[c, t, ch])
                    if n:
                        for b in range(s // P, (s + n + P - 1) // P):
                            entry_set[w].add((t, ch, b))
                    s += n
    # didx col order: per wave, sorted by (tile, chunk, block)
    wave_entries = []   # w -> {t: [(gcol, ch, b), ...]}
    wave_gw = []        # w -> (gw0, gw1)
    g = 0
    for w, wtiles in enumerate(waves):
        ents = sorted(entry_set[w])
        gw0 = g
        per_t = {t: [] for t in wtiles}
        for (t, ch, b) in ents:
            per_t[t].append((g, ch, b))
            g += 1
        wave_entries.append(per_t)
        wave_gw.append((gw0, g))
        for t in wtiles:
            assert per_t[t], f"tile {t} has no agg entries"
    NB_total = g

    idx16 = [np.zeros((N_CORES, P, S_ch[ch] // 16), np.int16) for ch in range(NCHUNK)]
    didx = np.full((N_CORES, P, NB_total), -1.0, np.float32)

    for c in range(N_CORES):
        srow, dl, cnt = per_core[c]
        ends = np.cumsum(cnt.reshape(-1))
        starts = ends - cnt.reshape(-1)
        idx_slots = [np.zeros(S_ch[ch], np.int16) for ch in range(NCHUNK)]
        didx_c = np.full((NB_total, P), -1.0, np.float32)
        for w, wtiles in enumerate(waves):
            for ch in range(NCHUNK):
                # concatenated segments for this core
                segs = []
                for t in wtiles:
                    k = t * NCHUNK + ch
                    segs.append((t, starts[k], ends[k]))
                cs = chunk_start[ch][w]
                pos = 0
                bounds = {}
                for t, e0, e1 in segs:
                    n = e1 - e0
                    if n:
                        idx_slots[ch][cs + pos:cs + pos + n] = srow[e0:e1].astype(np.int16)
                        bounds[t] = (pos, pos + n, e0)
                    pos += n
                if pos:
                    # pad region tail with the last valid index (dup descriptor)
                    idx_slots[ch][cs + pos:cs + blocks2[w, ch] * P] = idx_slots[ch][cs + pos - 1]
                # didx columns for this core's (t, block) coverage
                for t in wtiles:
                    if t not in bounds:
                        continue
                    p0, p1, e0 = bounds[t]
                    dloc = (dl[e0:e0 + (p1 - p0)] - t * P).astype(np.float32)
                    for (gcol, ech, b) in wave_entries[w][t]:
                        if ech != ch:
                            continue
                        lo = max(p0, b * P)
                        hi = min(p1, (b + 1) * P)
                        if lo < hi:
                            didx_c[gcol, lo - b * P:hi - b * P] = dloc[lo - p0:hi - p0]
        for ch in range(NCHUNK):
            w16 = idx_slots[ch].reshape(-1, 16).T  # [16, S/16]
            idx16[ch][c] = np.tile(w16, (8, 1))
        didx[c] = didx_c.T

    max_nb = int(blocks2.max())
    meta = {
        "waves": waves,
        "blocks2": blocks2,
        "S_ch": S_ch,
        "NB_total": NB_total,
        "chunk_start": chunk_start,
        "wave_entries": wave_entries,
        "wave_gw": wave_gw,
        "max_nb": max_nb,
    }
    return dinv, idx16, didx, meta


def _build_program(meta, has_bias):
    waves = meta["waves"]
    blocks2 = meta["blocks2"]
    S_ch = meta["S_ch"]
    NB_total = meta["NB_total"]
    chunk_start = meta["chunk_start"]
    wave_entries = meta["wave_entries"]
    wave_gw = meta["wave_gw"]
    max_nb = meta["max_nb"]

    nc = bacc.Bacc(
        "TRN2", target_bir_lowering=False, debug=False, num_devices=N_CORES,
        num_swdge_queues=NQUEUES, dynamic_dma_scratch_size=DMA_SCRATCH,
    )

    xT_io = nc.dram_tensor("xT", [F0, NODES_PER_CORE], dt.bfloat16, kind="ExternalInput").ap()
    sc_in_io = nc.dram_tensor("scIn", [P, NTILES], dt.float32, kind="ExternalInput").ap()   # dinv^2
    sc_out_io = nc.dram_tensor("scOut", [P, NTILES], dt.float32, kind="ExternalInput").ap() # dinv
    w1_io = nc.dram_tensor("W1sb", [P, F0], dt.bfloat16, kind="ExternalInput").ap()
    w2_io = nc.dram_tensor("W2pad", [P, P], dt.bfloat16, kind="ExternalInput").ap()
    w3_io = nc.dram_tensor("W3pad", [P, P], dt.bfloat16, kind="ExternalInput").ap()
    iota_io = nc.dram_tensor("iota", [P, P], dt.bfloat16, kind="ExternalInput").ap()
    iotaR_io = nc.dram_tensor("iotaR", [P, P * BB], dt.bfloat16, kind="ExternalInput").ap()
    idx_ios = [
        nc.dram_tensor(f"idx{ch}", [P, S_ch[ch] // 16], dt.int16, kind="ExternalInput").ap()
        for ch in range(NCHUNK)
    ]
    didx_io = nc.dram_tensor("didx", [P, NB_total], dt.bfloat16, kind="ExternalInput").ap()
    out_io = nc.dram_tensor("out3", [NODES_PER_CORE, 32], dt.float32, kind="ExternalOutput").ap()

    with tile.TileContext(nc) as tc:
        with (
            tc.tile_pool(name="const", bufs=1) as constp,
            tc.tile_pool(name="aT", bufs=1) as aTp,
            tc.tile_pool(name="hs", bufs=8) as hsp,
            tc.tile_pool(name="psum_d", bufs=2, space="PSUM") as psdp,
            tc.tile_pool(name="psum_a", bufs=4, space="PSUM") as psap,
            tc.tile_pool(name="psum_t", bufs=2, space="PSUM") as pstp,
            tc.tile_pool(name="dram", bufs=1, space="DRAM") as dramp,
        ):
            # ---- constants ----
            iota_t = constp.tile([P, P], dt.bfloat16)
            nc.sync.dma_start(iota_t[:], iota_io[:])
            sc_in = constp.tile([P, NTILES], dt.float32)
            nc.sync.dma_start(sc_in[:], sc_in_io[:])
            sc_out = constp.tile([P, NTILES], dt.float32)
            nc.sync.dma_start(sc_out[:], sc_out_io[:])
            w1_sb = constp.tile([P, F0], dt.bfloat16)
            nc.sync.dma_start(w1_sb[:], w1_io[:])
            w2_sb = constp.tile([P, P], dt.bfloat16)
            nc.sync.dma_start(w2_sb[:], w2_io[:])
            w3_sb = constp.tile([P, P], dt.bfloat16)
            nc.sync.dma_start(w3_sb[:], w3_io[:])
            didx_sb = constp.tile([P, NB_total], dt.bfloat16)
            nc.sync.dma_start(didx_sb[:], didx_io[:])
            iotaR = constp.tile([P, P, BB], dt.bfloat16)   # [p, j, g] = j
            nc.sync.dma_start(iotaR[:], iotaR_io[:])
            identb = constp.tile([P, P], dt.bfloat16)
            make_identity(nc, identb[:])

            # a_preT tiles (one set, reallocated per layer): [128 feats, qrows]
            current_aT = {}

            bounces = [
                dramp.tile([NODES_PER_CORE, FW], dt.bfloat16, tag=f"bnc{l}", name=f"bnc{l}")
                for l in range(3)
            ]
            tbls = [
                [dramp.tile([CH_ROWS[q], FW], dt.bfloat16, tag=f"tbl{l}q{q}",
                            name=f"tbl{l}q{q}", addr_space="Shared") for q in range(4)]
                for l in range(3)
            ]

            def q_of_t(t):
                return min(t // 25, 3)

            def dense_tile(l, t):
                """hs tile = a_pre @ W  (psum) -> bf16 hs tile -> bounce DRAM."""
                r = _rows(t)
                c0 = t * P
                ps = psdp.tile([P, P], dt.float32, space="PSUM", tag="pd")
                if l == 0:
                    h = 0 if t < 49 else 1
                    hb = 0 if t < 49 else 49 * P
                    nk = F0 // P
                    for k in range(nk):
                        xk = xhalves[h][k]
                        nc.tensor.matmul(
                            out=ps[:r, :], lhsT=xk[:, c0 - hb:c0 - hb + r],
                            rhs=w1_sb[:, k * P:(k + 1) * P],
                            start=(k == 0), stop=(k == nk - 1),
                        )
                else:
                    q = q_of_t(t)
                    off = (t - Q_TSTART[q]) * P
                    w = w2_sb if l == 1 else w3_sb
                    nc.tensor.matmul(
                        out=ps[:r, :], lhsT=current_aT[l][q][:, off:off + r], rhs=w[:],
                        start=True, stop=True,
                    )
                hs = hsp.tile([P, P], dt.bfloat16, tag="hs")
                nc.vector.tensor_copy(hs[:r, :], ps[:r, :])
                nc.sync.dma_start(bounces[l][c0:c0 + r, :], hs[:r, :])

            def emit_allgathers(l):
                for q in range(4):
                    r0 = Q_RSTART[q]
                    nc.gpsimd.collective_compute(
                        "AllGather", mybir.AluOpType.bypass,
                        replica_groups=[list(range(N_CORES))],
                        ins=[bounces[l][r0:r0 + Q_ROWS[q], :]],
                        outs=[tbls[l][q][:]],
                    )

            # ---- dense layer 0: staged x stripes, two node-halves ----
            xhalves = []
            with tc.tile_pool(name="xs", bufs=5) as xsp:
                for h in range(2):
                    hb = 0 if h == 0 else 49 * P
                    hc = 49 * P if h == 0 else NODES_PER_CORE - 49 * P
                    ks = []
                    for k in range(F0 // P):
                        xk = xsp.tile([P, 49 * P], dt.bfloat16, tag="xh")
                        nc.sync.dma_start(
                            xk[:, :hc], xT_io[k * P:(k + 1) * P, hb:hb + hc]
                        )
                        ks.append(xk)
                    xhalves.append(ks)
                for t in range(NTILES):
                    dense_tile(0, t)
                emit_allgathers(0)

            def agg_layer(l, idxp, msgp, selp, workp):
                ncols = NCOLS_MM[l]
                if l < 2:
                    current_aT[l + 1] = [
                        aTp.tile([P, Q_ROWS[q]], dt.bfloat16, tag=f"aq{q}",
                                 name=f"a{l + 1}q{q}")
                        for q in range(4)
                    ]
                for w, wtiles in enumerate(waves):
                    # gathers (one call per chunk, queue = chunk)
                    msg_tiles = {}
                    for ch in range(NCHUNK):
                        nb = int(blocks2[w, ch])
                        s0 = chunk_start[ch][w]
                        S = nb * P
                        iw = idxp.tile([P, max_nb * P // 16], dt.int16, tag="idx")
                        nc.sync.dma_start(iw[:, :S // 16], idx_ios[ch][:, s0 // 16:(s0 + S) // 16])
                        mt = msgp.tile([P, max_nb, FW], dt.bfloat16, tag="msg")
                        for b0 in range(0, nb, MAX_CALL_BLOCKS):
                            b1 = min(b0 + MAX_CALL_BLOCKS, nb)
                            Ssub = (b1 - b0) * P
                            nc.gpsimd.dma_gather(
                                out_ap=mt[:, b0:b1, :],
                                in_ap=tbls[l][ch][:],
                                idxs_ap=iw[:, b0 * P // 16:b1 * P // 16],
                                num_idxs=Ssub, num_idxs_reg=Ssub,
                                elem_size=FW, elem_step=FW,
                                single_packet=False,
                                queue_num=ch % NQUEUES,
                            )
                        msg_tiles[ch] = mt

                    # sel blocks: [slot, dst, block] layout so every operand's
                    # last dim is stride-1 (enables the DVE 2x mode)
                    gw0, gw1 = wave_gw[w]
                    sel_tiles = {}
                    for q0 in range(gw0, gw1, BB):
                        q1 = min(q0 + BB, gw1)
                        st = selp.tile([P, P, BB], dt.bfloat16, tag="sel")
                        nc.vector.tensor_tensor(
                            out=st[:, :, :q1 - q0],
                            in0=didx_sb[:, None, q0:q1].to_broadcast([P, P, q1 - q0]),
                            in1=iotaR[:, :, :q1 - q0],
                            op=mybir.AluOpType.is_equal,
                        )
                        for q in range(q0, q1):
                            sel_tiles[q] = (st, q - q0)

                    # per-tile matmul accumulation + post + (dense l+1)
                    for ti, t in enumerate(wtiles):
                        r = _rows(t)
                        c0 = t * P
                        ps = psap.tile([P, P], dt.float32, space="PSUM", tag="pa")
                        mms = [(gcol, ch, b) for (gcol, ch, b) in wave_entries[w][t]]
                        for i, (q, ch, col) in enumerate(mms):
                            st, j = sel_tiles[q]
                            nc.tensor.matmul(
                                out=ps[:r, :ncols], lhsT=st[:, :r, j],
                                rhs=msg_tiles[ch][:, col, :ncols],
                                start=(i == 0),
                                stop=(i == len(mms) - 1),
                            )
                        if l < 2:
                            outt = workp.tile([P, P], dt.bfloat16, tag="outb")
                            nc.scalar.activation(
                                outt[:r, :], ps[:r, :],
                                mybir.ActivationFunctionType.Relu,
                                scale=sc_in[:r, t:t + 1],
                            )
                            pst = pstp.tile([P, P], dt.bfloat16, space="PSUM", tag="pt")
                            nc.tensor.transpose(
                                out=pst[:, :r], in_=outt[:r, :], identity=identb[:r, :r]
                            )
                            q = q_of_t(t)
                            off = (t - Q_TSTART[q]) * P
                            nc.scalar.activation(
                                current_aT[l + 1][q][:, off:off + r], pst[:, :r],
                                mybir.ActivationFunctionType.Copy,
                            )
                            dense_tile(l + 1, t)
                        else:
                            outt = workp.tile([P, 32], dt.float32, tag="outf")
                            nc.scalar.activation(
                                outt[:r, :], ps[:r, :32],
                                mybir.ActivationFunctionType.Relu,
                                scale=sc_out[:r, t:t + 1],
                            )
                            nc.sync.dma_start(out_io[c0:c0 + r, :], outt[:r, :])
                if l < 2:
                    emit_allgathers(l + 1)

            with (
                tc.tile_pool(name="idxw", bufs=20) as idxp,
                tc.tile_pool(name="msgs", bufs=16) as msgp,
                tc.tile_pool(name="sel", bufs=8) as selp,
                tc.tile_pool(name="work", bufs=6) as workp,
            ):
                agg_layer(0, idxp, msgp, selp, workp)
                agg_layer(1, idxp, msgp, selp, workp)
                agg_layer(2, idxp, msgp, selp, workp)

    nc.compile()
    return nc


def _pack_inputs(x, dinv, W1, W2, W3, idx16, didx):
    iota_rep = np.tile(np.arange(P, dtype=np.float32)[None, :], (P, 1)).astype(BF)
    iotaR = np.tile(
        np.repeat(np.arange(P, dtype=np.float32), BB)[None, :], (P, 1)
    ).astype(BF)  # [p, j*BB + g] = j
    w1sb = np.zeros((P, F0), np.float32)
    for k in range(F0 // P):
        w1sb[:, k * P:(k + 1) * P] = W1[k * P:(k + 1) * P, :]
    w2pad = np.zeros((P, P), np.float32)
    w2pad[:, :64] = W2
    w3pad = np.zeros((P, P), np.float32)
    w3pad[:64, :32] = W3

    in_maps = []
    for c in range(N_CORES):
        lo = c * NODES_PER_CORE
        dv = dinv[lo:lo + NODES_PER_CORE]
        xs = x[lo:lo + NODES_PER_CORE].astype(np.float32) * dv[:, None]
        sc_in = np.ones((P, NTILES), np.float32)   # dinv^2 per tile col
        sc_out = np.ones((P, NTILES), np.float32)  # dinv per tile col
        for t in range(NTILES):
            r = _rows(t)
            sc_in[:r, t] = dv[t * P:t * P + r] ** 2
            sc_out[:r, t] = dv[t * P:t * P + r]
        in_maps.append({
            "xT": np.ascontiguousarray(xs.T).astype(BF),
            "scIn": sc_in,
            "scOut": sc_out,
            "W1sb": w1sb.astype(BF),
            "W2pad": w2pad.astype(BF),
            "W3pad": w3pad.astype(BF),
            "iota": iota_rep,
            "iotaR": iotaR,
            **{f"idx{ch}": idx16[ch][c] for ch in range(NCHUNK)},
            "didx": didx[c].astype(BF),
        })
    return in_maps


_TRACE = [False]          # set by test harness to request a profiled run
_LAST_RESULT = [None]     # BassKernelResults of the last run (for profiling)


def kernel(x, edge_index, batch, W1, b1, W2, b2, W3, b3, Wfc, bfc):
    x = np.asarray(x)
    edge_index = np.asarray(edge_index)
    batch = np.asarray(batch)
    W1, b1 = np.asarray(W1), np.asarray(b1)
    W2, b2 = np.asarray(W2), np.asarray(b2)
    W3, b3 = np.asarray(W3), np.asarray(b3)
    Wfc, bfc = np.asarray(Wfc), np.asarray(bfc)
    has_bias = bool(np.abs(b1).max() or np.abs(b2).max() or np.abs(b3).max())
    assert not has_bias, "nonzero conv biases not wired up in v2"

    dinv, idx16, didx, meta = _host_prep(edge_index.astype(np.int64))
    nc = _build_program(meta, has_bias)
    in_maps = _pack_inputs(x, dinv, W1, W2, W3, idx16, didx)
    res = run_bass_kernel_spmd(
        nc, in_maps, core_ids=list(range(N_CORES)), trace=_TRACE[0]
    )
    _LAST_RESULT[0] = res

    h3 = np.concatenate([res.results[c]["out3"][:, :32] for c in range(N_CORES)], axis=0)

    # host epilogue: segment max pool + FC + log_softmax (float64 for stability)
    pooled = np.full((N_GRAPHS, 32), -np.inf, np.float64)
    bnd = np.searchsorted(batch, np.arange(N_GRAPHS + 1))
    for g in range(N_GRAPHS):
        if bnd[g + 1] > bnd[g]:
            pooled[g] = h3[bnd[g]:bnd[g + 1]].max(axis=0)
    logits = pooled @ Wfc.astype(np.float64) + bfc.astype(np.float64)
    m = logits.max(axis=1, keepdims=True)
    lse = m + np.log(np.exp(logits - m).sum(axis=1, keepdims=True))
    return (logits - lse).astype(np.float32)


# revision 38
# speedup vs baseline: 1.9960x; 1.0287x over previous
"""GCN (3x GCNConv + global max pool + FC + log_softmax) on 8 Trainium2 NeuronCores.

v2 — pipelined rewrite of the working baseline:
  - 1D node partition: core c owns rows [12500c, 12500(c+1)).
  - dinv folded into inputs: x_pre = dinv*x on host; each conv's post is a single
    Act-engine relu with per-partition scale (dinv^2 for inner convs, dinv for the
    last) since relu(c*y) = c*relu(y) for c>0 and D(AW) = (DA)W.
  - dense(l): hs = a_pre @ W_l per local tile (PE, bf16) -> hs tile -> bounce DRAM.
    Quarter-granular AllGather (4 per layer, Shared outputs) so collectives start
    as soon as a quarter of the dense outputs lands and overlap with aggregation.
  - Aggregation: per-edge gather of table rows via gpsimd dma_gather (int16 idxs,
    4 src chunks = the AllGather quarters), then segment-sum via PE matmuls with
    per-128-edge one-hot selection matrices built on DVE, accumulated in PSUM per
    128-dst tile.  Gather idxs are src-sorted within groups; padding slots are -1
    (skipped by the DMA ucode).
  - dense(l+1) tiles are emitted immediately after each agg(l) tile so layers
    pipeline; AllGather triggers sit between layers' gather streams on gpsimd.
  - Pooling (segment max over graphs), tiny FC and log_softmax run on host.
"""

import sys

sys.path.insert(0, "/opt/trn_rl_repo")

import numpy as np
import ml_dtypes

import concourse.bass as bass
import concourse.bacc as bacc
import concourse.tile as tile
from concourse import mybir
from concourse.bass_utils import run_bass_kernel_spmd
from concourse.masks import make_identity

P = 128
N_NODES = 100000
N_EDGES = 1600000
N_GRAPHS = 64
N_CORES = 8
NODES_PER_CORE = N_NODES // N_CORES          # 12500
NTILES = (NODES_PER_CORE + P - 1) // P       # 98 (last tile 84 rows)
LAST_ROWS = NODES_PER_CORE - (NTILES - 1) * P  # 84
NCHUNK = 4
QT = [25, 25, 25, 23]                        # quarter tile counts
Q_TSTART = [0, 25, 50, 75]
Q_ROWS = [3200, 3200, 3200, 2900]            # per-core rows per quarter
Q_RSTART = [0, 3200, 6400, 9600]
CH_ROWS = [r * N_CORES for r in Q_ROWS]      # chunk rows: 25600,25600,25600,23200
WT = 4                                       # dst tiles per wave
F0 = 512
FW = 128                                     # table width (padded)
NCOLS_MM = [128, 64, 32]                     # agg matmul N per layer
MAX_CALL_BLOCKS = 47                         # ~6K idxs per dma_gather call
BB = 32                                      # sel blocks built per DVE instruction
NQUEUES = 4
DMA_SCRATCH = 16384
dt = mybir.dt
BF = ml_dtypes.bfloat16


def _rows(t):
    return LAST_ROWS if t == NTILES - 1 else P


def _host_prep(edge_index):
    """Build the shared (cross-core) aggregation schedule + per-core index data."""
    src = np.concatenate([edge_index[0], np.arange(N_NODES, dtype=np.int64)])
    dst = np.concatenate([edge_index[1], np.arange(N_NODES, dtype=np.int64)])
    deg = np.bincount(dst, minlength=N_NODES).astype(np.float32)
    dinv = (1.0 / np.sqrt(deg)).astype(np.float32)

    waves = [list(range(w, min(w + WT, NTILES))) for w in range(0, NTILES, WT)]

    # src -> (chunk, row-in-chunk): chunk = quarter of the owner's tile range
    s_core = src // NODES_PER_CORE
    s_loc = src % NODES_PER_CORE
    s_t = s_loc // P
    qs = np.searchsorted(np.array([25, 50, 75]), s_t, side="right")  # quarter id
    q_rows = np.array(Q_ROWS)
    q_rstart = np.array(Q_RSTART)
    s_row = s_core * q_rows[qs] + (s_loc - q_rstart[qs])

    per_core = []
    cnts = np.zeros((N_CORES, NTILES, NCHUNK), np.int64)
    d_core = dst // NODES_PER_CORE
    for c in range(N_CORES):
        m = d_core == c
        srow, sq = s_row[m], qs[m]
        dl = dst[m] - c * NODES_PER_CORE
        t = dl // P
        key = t * NCHUNK + sq
        o = np.lexsort((srow, key))          # group by (t,chunk), src-sorted within
        srow, dl, key = srow[o], dl[o], key[o]
        cnt = np.bincount(key, minlength=NTILES * NCHUNK).reshape(NTILES, NCHUNK)
        cnts[c] = cnt
        per_core.append((srow, dl, cnt))

    NW = len(waves)
    # contiguous per-(wave,chunk) slot packing: per-core tile segments are
    # concatenated without per-tile 128-padding; only the (w,ch) region is
    # padded to a block multiple (shared across cores via max).
    wcnt = np.zeros((N_CORES, NW, NCHUNK), np.int64)
    for c in range(N_CORES):
        for w, wtiles in enumerate(waves):
            for ch in range(NCHUNK):
                wcnt[c, w, ch] = cnts[c, wtiles, ch].sum()
    blocks2 = np.maximum((wcnt.max(axis=0) + P - 1) // P, 1)  # [NW, NCHUNK]

    chunk_start = [dict() for _ in range(NCHUNK)]   # [ch][w] -> slot offset
    for ch in range(NCHUNK):
        pos = 0
        for w in range(NW):
            chunk_start[ch][w] = pos
            pos += int(blocks2[w, ch]) * P
    S_ch = [int(blocks2[:, ch].sum() * P) for ch in range(NCHUNK)]

    # union over cores of (tile, block) pairs per (w, ch)
    entry_set = [set() for _ in range(NW)]
    for c in range(N_CORES):
        for w, wtiles in enumerate(waves):
            for ch in range(NCHUNK):
                s = 0
                for t in wtiles:
                    n = int(cnts[c, t, ch])
                    if n:
                        for b in range(s // P, (s + n + P - 1) // P):
                            entry_set[w].add((t, ch, b))
                    s += n
    # didx col order: per wave, sorted by (tile, chunk, block)
    wave_entries = []   # w -> {t: [(gcol, ch, b), ...]}
    wave_gw = []        # w -> (gw0, gw1)
    g = 0
    for w, wtiles in enumerate(waves):
        ents = sorted(entry_set[w])
        gw0 = g
        per_t = {t: [] for t in wtiles}
        for (t, ch, b) in ents:
            per_t[t].append((g, ch, b))
            g += 1
        wave_entries.append(per_t)
        wave_gw.append((gw0, g))
        for t in wtiles:
            assert per_t[t], f"tile {t} has no agg entries"
    NB_total = g

    idx16 = [np.zeros((N_CORES, P, S_ch[ch] // 16), np.int16) for ch in range(NCHUNK)]
    didx = np.full((N_CORES, P, NB_total), -1.0, np.float32)

    for c in range(N_CORES):
        srow, dl, cnt = per_core[c]
        ends = np.cumsum(cnt.reshape(-1))
        starts = ends - cnt.reshape(-1)
        idx_slots = [np.zeros(S_ch[ch], np.int16) for ch in range(NCHUNK)]
        didx_c = np.full((NB_total, P), -1.0, np.float32)
        for w, wtiles in enumerate(waves):
            for ch in range(NCHUNK):
                # concatenated segments for this core
                segs = []
                for t in wtiles:
                    k = t * NCHUNK + ch
                    segs.append((t, starts[k], ends[k]))
                cs = chunk_start[ch][w]
                pos = 0
                bounds = {}
                for t, e0, e1 in segs:
                    n = e1 - e0
                    if n:
                        idx_slots[ch][cs + pos:cs + pos + n] = srow[e0:e1].astype(np.int16)
                        bounds[t] = (pos, pos + n, e0)
                    pos += n
                if pos:
                    # pad region tail with the last valid index (dup descriptor)
                    idx_slots[ch][cs + pos:cs + blocks2[w, ch] * P] = idx_slots[ch][cs + pos - 1]
                # didx columns for this core's (t, block) coverage
                for t in wtiles:
                    if t not in bounds:
                        continue
                    p0, p1, e0 = bounds[t]
                    dloc = (dl[e0:e0 + (p1 - p0)] - t * P).astype(np.float32)
                    for (gcol, ech, b) in wave_entries[w][t]:
                        if ech != ch:
                            continue
                        lo = max(p0, b * P)
                        hi = min(p1, (b + 1) * P)
                        if lo < hi:
                            didx_c[gcol, lo - b * P:hi - b * P] = dloc[lo - p0:hi - p0]
        for ch in range(NCHUNK):
            w16 = idx_slots[ch].reshape(-1, 16).T  # [16, S/16]
            idx16[ch][c] = np.tile(w16, (8, 1))
        didx[c] = didx_c.T

    max_nb = int(blocks2.max())
    meta = {
        "waves": waves,
        "blocks2": blocks2,
        "S_ch": S_ch,
        "NB_total": NB_total,
        "chunk_start": chunk_start,
        "wave_entries": wave_entries,
        "wave_gw": wave_gw,
        "max_nb": max_nb,
    }
    return dinv, idx16, didx, meta


def _build_program(meta, has_bias):
    waves = meta["waves"]
    blocks2 = meta["blocks2"]
    S_ch = meta["S_ch"]
    NB_total = meta["NB_total"]
    chunk_start = meta["chunk_start"]
    wave_entries = meta["wave_entries"]
    wave_gw = meta["wave_gw"]
    max_nb = meta["max_nb"]

    nc = bacc.Bacc(
        "TRN2", target_bir_lowering=False, debug=False, num_devices=N_CORES,
        num_swdge_queues=NQUEUES, dynamic_dma_scratch_size=DMA_SCRATCH,
    )

    xT_io = nc.dram_tensor("xT", [F0, NODES_PER_CORE], dt.bfloat16, kind="ExternalInput").ap()
    sc_in_io = nc.dram_tensor("scIn", [P, NTILES], dt.float32, kind="ExternalInput").ap()   # dinv^2
    sc_out_io = nc.dram_tensor("scOut", [P, NTILES], dt.float32, kind="ExternalInput").ap() # dinv
    w1_io = nc.dram_tensor("W1sb", [P, F0], dt.bfloat16, kind="ExternalInput").ap()
    w2_io = nc.dram_tensor("W2pad", [P, P], dt.bfloat16, kind="ExternalInput").ap()
    w3_io = nc.dram_tensor("W3pad", [P, P], dt.bfloat16, kind="ExternalInput").ap()
    iota_io = nc.dram_tensor("iota", [P, P], dt.bfloat16, kind="ExternalInput").ap()
    iotaR_io = nc.dram_tensor("iotaR", [P, P * BB], dt.bfloat16, kind="ExternalInput").ap()
    idx_ios = [
        nc.dram_tensor(f"idx{ch}", [P, S_ch[ch] // 16], dt.int16, kind="ExternalInput").ap()
        for ch in range(NCHUNK)
    ]
    didx_io = nc.dram_tensor("didx", [P, NB_total], dt.bfloat16, kind="ExternalInput").ap()
    out_io = nc.dram_tensor("out3", [NODES_PER_CORE, 32], dt.float32, kind="ExternalOutput").ap()

    with tile.TileContext(nc) as tc:
        with (
            tc.tile_pool(name="const", bufs=1) as constp,
            tc.tile_pool(name="aT", bufs=1) as aTp,
            tc.tile_pool(name="hs", bufs=8) as hsp,
            tc.tile_pool(name="psum_d", bufs=2, space="PSUM") as psdp,
            tc.tile_pool(name="psum_a", bufs=4, space="PSUM") as psap,
            tc.tile_pool(name="psum_t", bufs=2, space="PSUM") as pstp,
            tc.tile_pool(name="dram", bufs=1, space="DRAM") as dramp,
        ):
            # ---- constants ----
            iota_t = constp.tile([P, P], dt.bfloat16)
            nc.sync.dma_start(iota_t[:], iota_io[:])
            sc_in = constp.tile([P, NTILES], dt.float32)
            nc.sync.dma_start(sc_in[:], sc_in_io[:])
            sc_out = constp.tile([P, NTILES], dt.float32)
            nc.sync.dma_start(sc_out[:], sc_out_io[:])
            w1_sb = constp.tile([P, F0], dt.bfloat16)
            nc.sync.dma_start(w1_sb[:], w1_io[:])
            w2_sb = constp.tile([P, P], dt.bfloat16)
            nc.sync.dma_start(w2_sb[:], w2_io[:])
            w3_sb = constp.tile([P, P], dt.bfloat16)
            nc.sync.dma_start(w3_sb[:], w3_io[:])
            didx_sb = constp.tile([P, NB_total], dt.bfloat16)
            nc.sync.dma_start(didx_sb[:], didx_io[:])
            iotaR = constp.tile([P, P, BB], dt.bfloat16)   # [p, j, g] = j
            nc.sync.dma_start(iotaR[:], iotaR_io[:])
            identb = constp.tile([P, P], dt.bfloat16)
            make_identity(nc, identb[:])

            # a_preT tiles (one set, reallocated per layer): [128 feats, qrows]
            current_aT = {}

            bounces = [
                dramp.tile([NODES_PER_CORE, FW], dt.bfloat16, tag=f"bnc{l}", name=f"bnc{l}")
                for l in range(3)
            ]
            tbls = [
                [dramp.tile([CH_ROWS[q], FW], dt.bfloat16, tag=f"tbl{l}q{q}",
                            name=f"tbl{l}q{q}", addr_space="Shared") for q in range(4)]
                for l in range(3)
            ]

            def q_of_t(t):
                return min(t // 25, 3)

            def dense_tile(l, t):
                """hs tile = a_pre @ W  (psum) -> bf16 hs tile -> bounce DRAM."""
                r = _rows(t)
                c0 = t * P
                ps = psdp.tile([P, P], dt.float32, space="PSUM", tag="pd")
                if l == 0:
                    h = 0 if t < 49 else 1
                    hb = 0 if t < 49 else 49 * P
                    nk = F0 // P
                    for k in range(nk):
                        xk = xhalves[h][k]
                        nc.tensor.matmul(
                            out=ps[:r, :], lhsT=xk[:, c0 - hb:c0 - hb + r],
                            rhs=w1_sb[:, k * P:(k + 1) * P],
                            start=(k == 0), stop=(k == nk - 1),
                        )
                else:
                    q = q_of_t(t)
                    off = (t - Q_TSTART[q]) * P
                    w = w2_sb if l == 1 else w3_sb
                    nc.tensor.matmul(
                        out=ps[:r, :], lhsT=current_aT[l][q][:, off:off + r], rhs=w[:],
                        start=True, stop=True,
                    )
                hs = hsp.tile([P, P], dt.bfloat16, tag="hs")
                nc.vector.tensor_copy(hs[:r, :], ps[:r, :])
                nc.scalar.dma_start(bounces[l][c0:c0 + r, :], hs[:r, :])

            def emit_allgathers(l):
                for q in range(4):
                    r0 = Q_RSTART[q]
                    nc.gpsimd.collective_compute(
                        "AllGather", mybir.AluOpType.bypass,
                        replica_groups=[list(range(N_CORES))],
                        ins=[bounces[l][r0:r0 + Q_ROWS[q], :]],
                        outs=[tbls[l][q][:]],
                    )

            # ---- dense layer 0: staged x stripes, two node-halves ----
            xhalves = []
            with tc.tile_pool(name="xs", bufs=5) as xsp:
                for h in range(2):
                    hb = 0 if h == 0 else 49 * P
                    hc = 49 * P if h == 0 else NODES_PER_CORE - 49 * P
                    ks = []
                    for k in range(F0 // P):
                        xk = xsp.tile([P, 49 * P], dt.bfloat16, tag="xh")
                        nc.scalar.dma_start(
                            xk[:, :hc], xT_io[k * P:(k + 1) * P, hb:hb + hc]
                        )
                        ks.append(xk)
                    xhalves.append(ks)
                for t in range(NTILES):
                    dense_tile(0, t)
                emit_allgathers(0)

            def agg_layer(l, idxp, msgp, selp, workp):
                ncols = NCOLS_MM[l]
                if l < 2:
                    current_aT[l + 1] = [
                        aTp.tile([P, Q_ROWS[q]], dt.bfloat16, tag=f"aq{q}",
                                 name=f"a{l + 1}q{q}")
                        for q in range(4)
                    ]
                # gather schedule: first 3 waves' ch3 deferred behind their
                # ch0-2 so the last AllGather's latency is covered with work
                NW = len(waves)
                sched = []
                defer = []
                for w in range(NW):
                    if w < 3:
                        sched += [(w, ch) for ch in range(NCHUNK - 1)]
                        defer.append((w, NCHUNK - 1))
                        if w == 2:
                            sched += defer
                    else:
                        sched += [(w, ch) for ch in range(NCHUNK)]
                msg_tiles_all = {}
                for (w, ch) in sched:
                    nb = int(blocks2[w, ch])
                    s0 = chunk_start[ch][w]
                    S = nb * P
                    iw = idxp.tile([P, max_nb * P // 16], dt.int16, tag="idx")
                    nc.sync.dma_start(iw[:, :S // 16], idx_ios[ch][:, s0 // 16:(s0 + S) // 16])
                    mt = msgp.tile([P, max_nb, FW], dt.bfloat16, tag="msg")
                    for b0 in range(0, nb, MAX_CALL_BLOCKS):
                        b1 = min(b0 + MAX_CALL_BLOCKS, nb)
                        Ssub = (b1 - b0) * P
                        nc.gpsimd.dma_gather(
                            out_ap=mt[:, b0:b1, :],
                            in_ap=tbls[l][ch][:],
                            idxs_ap=iw[:, b0 * P // 16:b1 * P // 16],
                            num_idxs=Ssub, num_idxs_reg=Ssub,
                            elem_size=FW, elem_step=FW,
                            single_packet=False,
                            queue_num=ch % NQUEUES,
                        )
                    msg_tiles_all[(w, ch)] = mt

                for w, wtiles in enumerate(waves):
                    # sel blocks: [slot, dst, block] layout so every operand's
                    # last dim is stride-1 (enables the DVE 2x mode)
                    gw0, gw1 = wave_gw[w]
                    sel_tiles = {}
                    for q0 in range(gw0, gw1, BB):
                        q1 = min(q0 + BB, gw1)
                        st = selp.tile([P, P, BB], dt.bfloat16, tag="sel")
                        nc.vector.tensor_tensor(
                            out=st[:, :, :q1 - q0],
                            in0=didx_sb[:, None, q0:q1].to_broadcast([P, P, q1 - q0]),
                            in1=iotaR[:, :, :q1 - q0],
                            op=mybir.AluOpType.is_equal,
                        )
                        for q in range(q0, q1):
                            sel_tiles[q] = (st, q - q0)

                    # per-tile matmul accumulation + post + (dense l+1)
                    for ti, t in enumerate(wtiles):
                        r = _rows(t)
                        c0 = t * P
                        ps = psap.tile([P, P], dt.float32, space="PSUM", tag="pa")
                        mms = [(gcol, ch, b) for (gcol, ch, b) in wave_entries[w][t]]
                        for i, (q, ch, col) in enumerate(mms):
                            st, j = sel_tiles[q]
                            nc.tensor.matmul(
                                out=ps[:r, :ncols], lhsT=st[:, :r, j],
                                rhs=msg_tiles_all[(w, ch)][:, col, :ncols],
                                start=(i == 0),
                                stop=(i == len(mms) - 1),
                            )
                        if l < 2:
                            outt = workp.tile([P, P], dt.bfloat16, tag="outb")
                            nc.scalar.activation(
                                outt[:r, :], ps[:r, :],
                                mybir.ActivationFunctionType.Relu,
                                scale=sc_in[:r, t:t + 1],
                            )
                            pst = pstp.tile([P, P], dt.bfloat16, space="PSUM", tag="pt")
                            nc.tensor.transpose(
                                out=pst[:, :r], in_=outt[:r, :], identity=identb[:r, :r]
                            )
                            q = q_of_t(t)
                            off = (t - Q_TSTART[q]) * P
                            nc.scalar.activation(
                                current_aT[l + 1][q][:, off:off + r], pst[:, :r],
                                mybir.ActivationFunctionType.Copy,
                            )
                            dense_tile(l + 1, t)
                        else:
                            outt = workp.tile([P, 32], dt.float32, tag="outf")
                            nc.scalar.activation(
                                outt[:r, :], ps[:r, :32],
                                mybir.ActivationFunctionType.Relu,
                                scale=sc_out[:r, t:t + 1],
                            )
                            nc.scalar.dma_start(out_io[c0:c0 + r, :], outt[:r, :])
                if l < 2:
                    emit_allgathers(l + 1)

            with (
                tc.tile_pool(name="idxw", bufs=20) as idxp,
                tc.tile_pool(name="msgs", bufs=16) as msgp,
                tc.tile_pool(name="sel", bufs=5) as selp,
                tc.tile_pool(name="work", bufs=6) as workp,
            ):
                agg_layer(0, idxp, msgp, selp, workp)
                agg_layer(1, idxp, msgp, selp, workp)
                agg_layer(2, idxp, msgp, selp, workp)

    nc.compile()
    return nc


def _pack_inputs(x, dinv, W1, W2, W3, idx16, didx):
    iota_rep = np.tile(np.arange(P, dtype=np.float32)[None, :], (P, 1)).astype(BF)
    iotaR = np.tile(
        np.repeat(np.arange(P, dtype=np.float32), BB)[None, :], (P, 1)
    ).astype(BF)  # [p, j*BB + g] = j
    w1sb = np.zeros((P, F0), np.float32)
    for k in range(F0 // P):
        w1sb[:, k * P:(k + 1) * P] = W1[k * P:(k + 1) * P, :]
    w2pad = np.zeros((P, P), np.float32)
    w2pad[:, :64] = W2
    w3pad = np.zeros((P, P), np.float32)
    w3pad[:64, :32] = W3

    in_maps = []
    for c in range(N_CORES):
        lo = c * NODES_PER_CORE
        dv = dinv[lo:lo + NODES_PER_CORE]
        xs = x[lo:lo + NODES_PER_CORE].astype(np.float32) * dv[:, None]
        sc_in = np.ones((P, NTILES), np.float32)   # dinv^2 per tile col
        sc_out = np.ones((P, NTILES), np.float32)  # dinv per tile col
        for t in range(NTILES):
            r = _rows(t)
            sc_in[:r, t] = dv[t * P:t * P + r] ** 2
            sc_out[:r, t] = dv[t * P:t * P + r]
        in_maps.append({
            "xT": np.ascontiguousarray(xs.T).astype(BF),
            "scIn": sc_in,
            "scOut": sc_out,
            "W1sb": w1sb.astype(BF),
            "W2pad": w2pad.astype(BF),
            "W3pad": w3pad.astype(BF),
            "iota": iota_rep,
            "iotaR": iotaR,
            **{f"idx{ch}": idx16[ch][c] for ch in range(NCHUNK)},
            "didx": didx[c].astype(BF),
        })
    return in_maps


_TRACE = [False]          # set by test harness to request a profiled run
_LAST_RESULT = [None]     # BassKernelResults of the last run (for profiling)


def kernel(x, edge_index, batch, W1, b1, W2, b2, W3, b3, Wfc, bfc):
    x = np.asarray(x)
    edge_index = np.asarray(edge_index)
    batch = np.asarray(batch)
    W1, b1 = np.asarray(W1), np.asarray(b1)
    W2, b2 = np.asarray(W2), np.asarray(b2)
    W3, b3 = np.asarray(W3), np.asarray(b3)
    Wfc, bfc = np.asarray(Wfc), np.asarray(bfc)
    has_bias = bool(np.abs(b1).max() or np.abs(b2).max() or np.abs(b3).max())
    assert not has_bias, "nonzero conv biases not wired up in v2"

    dinv, idx16, didx, meta = _host_prep(edge_index.astype(np.int64))
    nc = _build_program(meta, has_bias)
    in_maps = _pack_inputs(x, dinv, W1, W2, W3, idx16, didx)
    res = run_bass_kernel_spmd(
        nc, in_maps, core_ids=list(range(N_CORES)), trace=_TRACE[0]
    )
    _LAST_RESULT[0] = res

    h3 = np.concatenate([res.results[c]["out3"][:, :32] for c in range(N_CORES)], axis=0)

    # host epilogue: segment max pool + FC + log_softmax (float64 for stability)
    pooled = np.full((N_GRAPHS, 32), -np.inf, np.float64)
    bnd = np.searchsorted(batch, np.arange(N_GRAPHS + 1))
    for g in range(N_GRAPHS):
        if bnd[g + 1] > bnd[g]:
            pooled[g] = h3[bnd[g]:bnd[g + 1]].max(axis=0)
    logits = pooled @ Wfc.astype(np.float64) + bfc.astype(np.float64)
    m = logits.max(axis=1, keepdims=True)
    lse = m + np.log(np.exp(logits - m).sum(axis=1, keepdims=True))
    return (logits - lse).astype(np.float32)


# revision 41
# speedup vs baseline: 2.0884x; 1.0463x over previous
"""GCN (3x GCNConv + global max pool + FC + log_softmax) on 8 Trainium2 NeuronCores.

v2 — pipelined rewrite of the working baseline:
  - 1D node partition: core c owns rows [12500c, 12500(c+1)).
  - dinv folded into inputs: x_pre = dinv*x on host; each conv's post is a single
    Act-engine relu with per-partition scale (dinv^2 for inner convs, dinv for the
    last) since relu(c*y) = c*relu(y) for c>0 and D(AW) = (DA)W.
  - dense(l): hs = a_pre @ W_l per local tile (PE, bf16) -> hs tile -> bounce DRAM.
    Quarter-granular AllGather (4 per layer, Shared outputs) so collectives start
    as soon as a quarter of the dense outputs lands and overlap with aggregation.
  - Aggregation: per-edge gather of table rows via gpsimd dma_gather (int16 idxs,
    4 src chunks = the AllGather quarters), then segment-sum via PE matmuls with
    per-128-edge one-hot selection matrices built on DVE, accumulated in PSUM per
    128-dst tile.  Gather idxs are src-sorted within groups; padding slots are -1
    (skipped by the DMA ucode).
  - dense(l+1) tiles are emitted immediately after each agg(l) tile so layers
    pipeline; AllGather triggers sit between layers' gather streams on gpsimd.
  - Pooling (segment max over graphs), tiny FC and log_softmax run on host.
"""

import sys

sys.path.insert(0, "/opt/trn_rl_repo")

import numpy as np
import ml_dtypes

import concourse.bass as bass
import concourse.bacc as bacc
import concourse.tile as tile
from concourse import mybir
from concourse.bass_utils import run_bass_kernel_spmd
from concourse.masks import make_identity

P = 128
N_NODES = 100000
N_EDGES = 1600000
N_GRAPHS = 64
N_CORES = 8
NODES_PER_CORE = N_NODES // N_CORES          # 12500
NTILES = (NODES_PER_CORE + P - 1) // P       # 98 (last tile 84 rows)
LAST_ROWS = NODES_PER_CORE - (NTILES - 1) * P  # 84
NCHUNK = 4
QT = [25, 25, 25, 23]                        # quarter tile counts
Q_TSTART = [0, 25, 50, 75]
Q_ROWS = [3200, 3200, 3200, 2900]            # per-core rows per quarter
Q_RSTART = [0, 3200, 6400, 9600]
CH_ROWS = [r * N_CORES for r in Q_ROWS]      # chunk rows: 25600,25600,25600,23200
WT = 4                                       # dst tiles per wave
F0 = 512
FW = 128                                     # table width (padded)
NCOLS_MM = [128, 64, 32]                     # agg matmul N per layer
MAX_CALL_BLOCKS = 47                         # ~6K idxs per dma_gather call
BB = 32                                      # sel blocks built per DVE instruction
NQUEUES = 4
DMA_SCRATCH = 16384
dt = mybir.dt
BF = ml_dtypes.bfloat16


def _rows(t):
    return LAST_ROWS if t == NTILES - 1 else P


def _host_prep(edge_index):
    """Build the shared (cross-core) aggregation schedule + per-core index data."""
    src = np.concatenate([edge_index[0], np.arange(N_NODES, dtype=np.int64)])
    dst = np.concatenate([edge_index[1], np.arange(N_NODES, dtype=np.int64)])
    deg = np.bincount(dst, minlength=N_NODES).astype(np.float32)
    dinv = (1.0 / np.sqrt(deg)).astype(np.float32)

    waves = [list(range(w, min(w + WT, NTILES))) for w in range(0, NTILES, WT)]

    # src -> (chunk, row-in-chunk): chunk = quarter of the owner's tile range
    s_core = src // NODES_PER_CORE
    s_loc = src % NODES_PER_CORE
    s_t = s_loc // P
    qs = np.searchsorted(np.array([25, 50, 75]), s_t, side="right")  # quarter id
    q_rows = np.array(Q_ROWS)
    q_rstart = np.array(Q_RSTART)
    s_row = s_core * q_rows[qs] + (s_loc - q_rstart[qs])

    per_core = []
    cnts = np.zeros((N_CORES, NTILES, NCHUNK), np.int64)
    d_core = dst // NODES_PER_CORE
    for c in range(N_CORES):
        m = d_core == c
        srow, sq = s_row[m], qs[m]
        dl = dst[m] - c * NODES_PER_CORE
        t = dl // P
        key = t * NCHUNK + sq
        o = np.lexsort((srow, key))          # group by (t,chunk), src-sorted within
        srow, dl, key = srow[o], dl[o], key[o]
        cnt = np.bincount(key, minlength=NTILES * NCHUNK).reshape(NTILES, NCHUNK)
        cnts[c] = cnt
        per_core.append((srow, dl, cnt))

    NW = len(waves)
    # contiguous per-(wave,chunk) slot packing: per-core tile segments are
    # concatenated without per-tile 128-padding; only the (w,ch) region is
    # padded to a block multiple (shared across cores via max).
    wcnt = np.zeros((N_CORES, NW, NCHUNK), np.int64)
    for c in range(N_CORES):
        for w, wtiles in enumerate(waves):
            for ch in range(NCHUNK):
                wcnt[c, w, ch] = cnts[c, wtiles, ch].sum()
    blocks2 = np.maximum((wcnt.max(axis=0) + P - 1) // P, 1)  # [NW, NCHUNK]

    chunk_start = [dict() for _ in range(NCHUNK)]   # [ch][w] -> slot offset
    for ch in range(NCHUNK):
        pos = 0
        for w in range(NW):
            chunk_start[ch][w] = pos
            pos += int(blocks2[w, ch]) * P
    S_ch = [int(blocks2[:, ch].sum() * P) for ch in range(NCHUNK)]

    # union over cores of (tile, block) pairs per (w, ch)
    entry_set = [set() for _ in range(NW)]
    for c in range(N_CORES):
        for w, wtiles in enumerate(waves):
            for ch in range(NCHUNK):
                s = 0
                for t in wtiles:
                    n = int(cnts[c, t, ch])
                    if n:
                        for b in range(s // P, (s + n + P - 1) // P):
                            entry_set[w].add((t, ch, b))
                    s += n
    # didx col order: per wave, sorted by (tile, chunk, block)
    wave_entries = []   # w -> {t: [(gcol, ch, b), ...]}
    wave_gw = []        # w -> (gw0, gw1)
    g = 0
    for w, wtiles in enumerate(waves):
        ents = sorted(entry_set[w])
        gw0 = g
        per_t = {t: [] for t in wtiles}
        for (t, ch, b) in ents:
            per_t[t].append((g, ch, b))
            g += 1
        wave_entries.append(per_t)
        wave_gw.append((gw0, g))
        for t in wtiles:
            assert per_t[t], f"tile {t} has no agg entries"
    NB_total = g

    idx16 = [np.zeros((N_CORES, P, S_ch[ch] // 16), np.int16) for ch in range(NCHUNK)]
    didx = np.full((N_CORES, P, NB_total), -1.0, np.float32)

    for c in range(N_CORES):
        srow, dl, cnt = per_core[c]
        ends = np.cumsum(cnt.reshape(-1))
        starts = ends - cnt.reshape(-1)
        idx_slots = [np.zeros(S_ch[ch], np.int16) for ch in range(NCHUNK)]
        didx_c = np.full((NB_total, P), -1.0, np.float32)
        for w, wtiles in enumerate(waves):
            for ch in range(NCHUNK):
                # concatenated segments for this core
                segs = []
                for t in wtiles:
                    k = t * NCHUNK + ch
                    segs.append((t, starts[k], ends[k]))
                cs = chunk_start[ch][w]
                pos = 0
                bounds = {}
                for t, e0, e1 in segs:
                    n = e1 - e0
                    if n:
                        idx_slots[ch][cs + pos:cs + pos + n] = srow[e0:e1].astype(np.int16)
                        bounds[t] = (pos, pos + n, e0)
                    pos += n
                if pos:
                    # pad region tail with the last valid index (dup descriptor)
                    idx_slots[ch][cs + pos:cs + blocks2[w, ch] * P] = idx_slots[ch][cs + pos - 1]
                # didx columns for this core's (t, block) coverage
                for t in wtiles:
                    if t not in bounds:
                        continue
                    p0, p1, e0 = bounds[t]
                    dloc = (dl[e0:e0 + (p1 - p0)] - t * P).astype(np.float32)
                    for (gcol, ech, b) in wave_entries[w][t]:
                        if ech != ch:
                            continue
                        lo = max(p0, b * P)
                        hi = min(p1, (b + 1) * P)
                        if lo < hi:
                            didx_c[gcol, lo - b * P:hi - b * P] = dloc[lo - p0:hi - p0]
        for ch in range(NCHUNK):
            w16 = idx_slots[ch].reshape(-1, 16).T  # [16, S/16]
            idx16[ch][c] = np.tile(w16, (8, 1))
        didx[c] = didx_c.T

    max_nb = int(blocks2.max())
    meta = {
        "waves": waves,
        "blocks2": blocks2,
        "S_ch": S_ch,
        "NB_total": NB_total,
        "chunk_start": chunk_start,
        "wave_entries": wave_entries,
        "wave_gw": wave_gw,
        "max_nb": max_nb,
    }
    return dinv, idx16, didx, meta


def _build_program(meta, has_bias):
    waves = meta["waves"]
    blocks2 = meta["blocks2"]
    S_ch = meta["S_ch"]
    NB_total = meta["NB_total"]
    chunk_start = meta["chunk_start"]
    wave_entries = meta["wave_entries"]
    wave_gw = meta["wave_gw"]
    max_nb = meta["max_nb"]

    nc = bacc.Bacc(
        "TRN2", target_bir_lowering=False, debug=False, num_devices=N_CORES,
        num_swdge_queues=NQUEUES, dynamic_dma_scratch_size=DMA_SCRATCH,
    )

    xT_io = nc.dram_tensor("xT", [F0, NODES_PER_CORE], dt.bfloat16, kind="ExternalInput").ap()
    sc_in_io = nc.dram_tensor("scIn", [P, NTILES], dt.float32, kind="ExternalInput").ap()   # dinv^2
    sc_out_io = nc.dram_tensor("scOut", [P, NTILES], dt.float32, kind="ExternalInput").ap() # dinv
    w1_io = nc.dram_tensor("W1sb", [P, F0], dt.bfloat16, kind="ExternalInput").ap()
    w2_io = nc.dram_tensor("W2pad", [P, P], dt.bfloat16, kind="ExternalInput").ap()
    w3_io = nc.dram_tensor("W3pad", [P, P], dt.bfloat16, kind="ExternalInput").ap()
    iota_io = nc.dram_tensor("iota", [P, P], dt.bfloat16, kind="ExternalInput").ap()
    iotaR_io = nc.dram_tensor("iotaR", [P, P * BB], dt.bfloat16, kind="ExternalInput").ap()
    idx_ios = [
        nc.dram_tensor(f"idx{ch}", [P, S_ch[ch] // 16], dt.int16, kind="ExternalInput").ap()
        for ch in range(NCHUNK)
    ]
    didx_io = nc.dram_tensor("didx", [P, NB_total], dt.bfloat16, kind="ExternalInput").ap()
    out_io = nc.dram_tensor("out3", [NODES_PER_CORE, 32], dt.float32, kind="ExternalOutput").ap()

    with tile.TileContext(nc) as tc:
        with (
            tc.tile_pool(name="const", bufs=1) as constp,
            tc.tile_pool(name="aT", bufs=1) as aTp,
            tc.tile_pool(name="hs", bufs=8) as hsp,
            tc.tile_pool(name="psum_d", bufs=2, space="PSUM") as psdp,
            tc.tile_pool(name="psum_a", bufs=4, space="PSUM") as psap,
            tc.tile_pool(name="psum_t", bufs=2, space="PSUM") as pstp,
            tc.tile_pool(name="dram", bufs=1, space="DRAM") as dramp,
        ):
            # ---- constants ----
            iota_t = constp.tile([P, P], dt.bfloat16)
            nc.sync.dma_start(iota_t[:], iota_io[:])
            sc_in = constp.tile([P, NTILES], dt.float32)
            nc.sync.dma_start(sc_in[:], sc_in_io[:])
            sc_out = constp.tile([P, NTILES], dt.float32)
            nc.sync.dma_start(sc_out[:], sc_out_io[:])
            w1_sb = constp.tile([P, F0], dt.bfloat16)
            nc.sync.dma_start(w1_sb[:], w1_io[:])
            w2_sb = constp.tile([P, P], dt.bfloat16)
            nc.sync.dma_start(w2_sb[:], w2_io[:])
            w3_sb = constp.tile([P, P], dt.bfloat16)
            nc.sync.dma_start(w3_sb[:], w3_io[:])
            didx_sb = constp.tile([P, NB_total], dt.bfloat16)
            nc.sync.dma_start(didx_sb[:], didx_io[:])
            iotaR = constp.tile([P, P, BB], dt.bfloat16)   # [p, j, g] = j
            nc.sync.dma_start(iotaR[:], iotaR_io[:])
            identb = constp.tile([P, P], dt.bfloat16)
            make_identity(nc, identb[:])

            # a_preT tiles (one set, reallocated per layer): [128 feats, qrows]
            current_aT = {}

            bounces = [
                dramp.tile([NODES_PER_CORE, FW], dt.bfloat16, tag=f"bnc{l}", name=f"bnc{l}")
                for l in range(3)
            ]
            tbls = [
                [dramp.tile([CH_ROWS[q], FW], dt.bfloat16, tag=f"tbl{l}q{q}",
                            name=f"tbl{l}q{q}", addr_space="Shared") for q in range(4)]
                for l in range(3)
            ]

            def q_of_t(t):
                return min(t // 25, 3)

            def dense_tile(l, t):
                """hs tile = a_pre @ W  (psum) -> bf16 hs tile -> bounce DRAM."""
                r = _rows(t)
                c0 = t * P
                ps = psdp.tile([P, P], dt.float32, space="PSUM", tag="pd")
                if l == 0:
                    h = 0 if t < 49 else 1
                    hb = 0 if t < 49 else 49 * P
                    nk = F0 // P
                    for k in range(nk):
                        xk = xhalves[h][k]
                        nc.tensor.matmul(
                            out=ps[:r, :], lhsT=xk[:, c0 - hb:c0 - hb + r],
                            rhs=w1_sb[:, k * P:(k + 1) * P],
                            start=(k == 0), stop=(k == nk - 1),
                        )
                else:
                    q = q_of_t(t)
                    off = (t - Q_TSTART[q]) * P
                    w = w2_sb if l == 1 else w3_sb
                    nc.tensor.matmul(
                        out=ps[:r, :], lhsT=current_aT[l][q][:, off:off + r], rhs=w[:],
                        start=True, stop=True,
                    )
                hs = hsp.tile([P, P], dt.bfloat16, tag="hs")
                nc.vector.tensor_copy(hs[:r, :], ps[:r, :])
                nc.scalar.dma_start(bounces[l][c0:c0 + r, :], hs[:r, :])

            def emit_ag(l, q):
                r0 = Q_RSTART[q]
                nc.gpsimd.collective_compute(
                    "AllGather", mybir.AluOpType.bypass,
                    replica_groups=[list(range(N_CORES))],
                    ins=[bounces[l][r0:r0 + Q_ROWS[q], :]],
                    outs=[tbls[l][q][:]],
                )

            def emit_allgathers(l):
                for q in range(4):
                    emit_ag(l, q)

            # ---- dense layer 0: staged x stripes, two node-halves ----
            xhalves = []
            with tc.tile_pool(name="xs", bufs=5) as xsp:
                for h in range(2):
                    hb = 0 if h == 0 else 49 * P
                    hc = 49 * P if h == 0 else NODES_PER_CORE - 49 * P
                    ks = []
                    for k in range(F0 // P):
                        xk = xsp.tile([P, 49 * P], dt.bfloat16, tag="xh")
                        eng = nc.scalar if k % 2 == 0 else nc.sync
                        eng.dma_start(
                            xk[:, :hc], xT_io[k * P:(k + 1) * P, hb:hb + hc]
                        )
                        ks.append(xk)
                    xhalves.append(ks)
                for t in range(NTILES):
                    dense_tile(0, t)
                emit_allgathers(0)

            def agg_layer(l, idxp, msgp, selp, workp):
                ncols = NCOLS_MM[l]
                if l < 2:
                    current_aT[l + 1] = [
                        aTp.tile([P, Q_ROWS[q]], dt.bfloat16, tag=f"aq{q}",
                                 name=f"a{l + 1}q{q}")
                        for q in range(4)
                    ]
                # gather schedule: first 3 waves' ch3 deferred behind their
                # ch0-2 so the last AllGather's latency is covered with work
                NW = len(waves)
                sched = []
                defer = []
                for w in range(NW):
                    if w < 3:
                        sched += [(w, ch) for ch in range(NCHUNK - 1)]
                        defer.append((w, NCHUNK - 1))
                        if w == 2:
                            sched += defer
                    else:
                        sched += [(w, ch) for ch in range(NCHUNK)]
                msg_tiles_all = {}
                sp = [0]  # pointer into sched

                def emit_gathers_due(wlimit):
                    while sp[0] < len(sched) and sched[sp[0]][0] <= wlimit:
                        w, ch = sched[sp[0]]
                        sp[0] += 1
                        nb = int(blocks2[w, ch])
                        s0 = chunk_start[ch][w]
                        S = nb * P
                        iw = idxp.tile([P, max_nb * P // 16], dt.int16, tag="idx")
                        nc.sync.dma_start(iw[:, :S // 16], idx_ios[ch][:, s0 // 16:(s0 + S) // 16])
                        mt = msgp.tile([P, max_nb, FW], dt.bfloat16, tag="msg")
                        for b0 in range(0, nb, MAX_CALL_BLOCKS):
                            b1 = min(b0 + MAX_CALL_BLOCKS, nb)
                            Ssub = (b1 - b0) * P
                            nc.gpsimd.dma_gather(
                                out_ap=mt[:, b0:b1, :],
                                in_ap=tbls[l][ch][:],
                                idxs_ap=iw[:, b0 * P // 16:b1 * P // 16],
                                num_idxs=Ssub, num_idxs_reg=Ssub,
                                elem_size=FW, elem_step=FW,
                                single_packet=False,
                                queue_num=ch % NQUEUES,
                            )
                        msg_tiles_all[(w, ch)] = mt

                # quarter q of the next layer is fully produced after this wave
                qend = {6: 0, 12: 1, 18: 2} if l < 2 else {}
                for w, wtiles in enumerate(waves):
                    emit_gathers_due(w + 3)
                    # sel blocks: [slot, dst, block] layout so every operand's
                    # last dim is stride-1 (enables the DVE 2x mode)
                    gw0, gw1 = wave_gw[w]
                    sel_tiles = {}
                    for q0 in range(gw0, gw1, BB):
                        q1 = min(q0 + BB, gw1)
                        st = selp.tile([P, P, BB], dt.bfloat16, tag="sel")
                        nc.vector.tensor_tensor(
                            out=st[:, :, :q1 - q0],
                            in0=didx_sb[:, None, q0:q1].to_broadcast([P, P, q1 - q0]),
                            in1=iotaR[:, :, :q1 - q0],
                            op=mybir.AluOpType.is_equal,
                        )
                        for q in range(q0, q1):
                            sel_tiles[q] = (st, q - q0)

                    # per-tile matmul accumulation + post + (dense l+1)
                    for ti, t in enumerate(wtiles):
                        r = _rows(t)
                        c0 = t * P
                        ps = psap.tile([P, P], dt.float32, space="PSUM", tag="pa")
                        mms = [(gcol, ch, b) for (gcol, ch, b) in wave_entries[w][t]]
                        for i, (q, ch, col) in enumerate(mms):
                            st, j = sel_tiles[q]
                            nc.tensor.matmul(
                                out=ps[:r, :ncols], lhsT=st[:, :r, j],
                                rhs=msg_tiles_all[(w, ch)][:, col, :ncols],
                                start=(i == 0),
                                stop=(i == len(mms) - 1),
                            )
                        if l < 2:
                            outt = workp.tile([P, P], dt.bfloat16, tag="outb")
                            nc.scalar.activation(
                                outt[:r, :], ps[:r, :],
                                mybir.ActivationFunctionType.Relu,
                                scale=sc_in[:r, t:t + 1],
                            )
                            pst = pstp.tile([P, P], dt.bfloat16, space="PSUM", tag="pt")
                            nc.tensor.transpose(
                                out=pst[:, :r], in_=outt[:r, :], identity=identb[:r, :r]
                            )
                            q = q_of_t(t)
                            off = (t - Q_TSTART[q]) * P
                            nc.scalar.activation(
                                current_aT[l + 1][q][:, off:off + r], pst[:, :r],
                                mybir.ActivationFunctionType.Copy,
                            )
                            dense_tile(l + 1, t)
                        else:
                            outt = workp.tile([P, 32], dt.float32, tag="outf")
                            nc.scalar.activation(
                                outt[:r, :], ps[:r, :32],
                                mybir.ActivationFunctionType.Relu,
                                scale=sc_out[:r, t:t + 1],
                            )
                            nc.scalar.dma_start(out_io[c0:c0 + r, :], outt[:r, :])
                    if w in qend:
                        emit_ag(l + 1, qend[w])
                if l < 2:
                    emit_ag(l + 1, 3)

            with (
                tc.tile_pool(name="idxw", bufs=20) as idxp,
                tc.tile_pool(name="msgs", bufs=16) as msgp,
                tc.tile_pool(name="sel", bufs=5) as selp,
                tc.tile_pool(name="work", bufs=6) as workp,
            ):
                agg_layer(0, idxp, msgp, selp, workp)
                agg_layer(1, idxp, msgp, selp, workp)
                agg_layer(2, idxp, msgp, selp, workp)

    nc.compile()
    return nc


def _pack_inputs(x, dinv, W1, W2, W3, idx16, didx):
    iota_rep = np.tile(np.arange(P, dtype=np.float32)[None, :], (P, 1)).astype(BF)
    iotaR = np.tile(
        np.repeat(np.arange(P, dtype=np.float32), BB)[None, :], (P, 1)
    ).astype(BF)  # [p, j*BB + g] = j
    w1sb = np.zeros((P, F0), np.float32)
    for k in range(F0 // P):
        w1sb[:, k * P:(k + 1) * P] = W1[k * P:(k + 1) * P, :]
    w2pad = np.zeros((P, P), np.float32)
    w2pad[:, :64] = W2
    w3pad = np.zeros((P, P), np.float32)
    w3pad[:64, :32] = W3

    in_maps = []
    for c in range(N_CORES):
        lo = c * NODES_PER_CORE
        dv = dinv[lo:lo + NODES_PER_CORE]
        xs = x[lo:lo + NODES_PER_CORE].astype(np.float32) * dv[:, None]
        sc_in = np.ones((P, NTILES), np.float32)   # dinv^2 per tile col
        sc_out = np.ones((P, NTILES), np.float32)  # dinv per tile col
        for t in range(NTILES):
            r = _rows(t)
            sc_in[:r, t] = dv[t * P:t * P + r] ** 2
            sc_out[:r, t] = dv[t * P:t * P + r]
        in_maps.append({
            "xT": np.ascontiguousarray(xs.T).astype(BF),
            "scIn": sc_in,
            "scOut": sc_out,
            "W1sb": w1sb.astype(BF),
            "W2pad": w2pad.astype(BF),
            "W3pad": w3pad.astype(BF),
            "iota": iota_rep,
            "iotaR": iotaR,
            **{f"idx{ch}": idx16[ch][c] for ch in range(NCHUNK)},
            "didx": didx[c].astype(BF),
        })
    return in_maps


_TRACE = [False]          # set by test harness to request a profiled run
_LAST_RESULT = [None]     # BassKernelResults of the last run (for profiling)


def kernel(x, edge_index, batch, W1, b1, W2, b2, W3, b3, Wfc, bfc):
    x = np.asarray(x)
    edge_index = np.asarray(edge_index)
    batch = np.asarray(batch)
    W1, b1 = np.asarray(W1), np.asarray(b1)
    W2, b2 = np.asarray(W2), np.asarray(b2)
    W3, b3 = np.asarray(W3), np.asarray(b3)
    Wfc, bfc = np.asarray(Wfc), np.asarray(bfc)
    has_bias = bool(np.abs(b1).max() or np.abs(b2).max() or np.abs(b3).max())
    assert not has_bias, "nonzero conv biases not wired up in v2"

    dinv, idx16, didx, meta = _host_prep(edge_index.astype(np.int64))
    nc = _build_program(meta, has_bias)
    in_maps = _pack_inputs(x, dinv, W1, W2, W3, idx16, didx)
    res = run_bass_kernel_spmd(
        nc, in_maps, core_ids=list(range(N_CORES)), trace=_TRACE[0]
    )
    _LAST_RESULT[0] = res

    h3 = np.concatenate([res.results[c]["out3"][:, :32] for c in range(N_CORES)], axis=0)

    # host epilogue: segment max pool + FC + log_softmax (float64 for stability)
    pooled = np.full((N_GRAPHS, 32), -np.inf, np.float64)
    bnd = np.searchsorted(batch, np.arange(N_GRAPHS + 1))
    for g in range(N_GRAPHS):
        if bnd[g + 1] > bnd[g]:
            pooled[g] = h3[bnd[g]:bnd[g + 1]].max(axis=0)
    logits = pooled @ Wfc.astype(np.float64) + bfc.astype(np.float64)
    m = logits.max(axis=1, keepdims=True)
    lse = m + np.log(np.exp(logits - m).sum(axis=1, keepdims=True))
    return (logits - lse).astype(np.float32)


# revision 43
# speedup vs baseline: 2.0943x; 1.0028x over previous
"""GCN (3x GCNConv + global max pool + FC + log_softmax) on 8 Trainium2 NeuronCores.

v2 — pipelined rewrite of the working baseline:
  - 1D node partition: core c owns rows [12500c, 12500(c+1)).
  - dinv folded into inputs: x_pre = dinv*x on host; each conv's post is a single
    Act-engine relu with per-partition scale (dinv^2 for inner convs, dinv for the
    last) since relu(c*y) = c*relu(y) for c>0 and D(AW) = (DA)W.
  - dense(l): hs = a_pre @ W_l per local tile (PE, bf16) -> hs tile -> bounce DRAM.
    Quarter-granular AllGather (4 per layer, Shared outputs) so collectives start
    as soon as a quarter of the dense outputs lands and overlap with aggregation.
  - Aggregation: per-edge gather of table rows via gpsimd dma_gather (int16 idxs,
    4 src chunks = the AllGather quarters), then segment-sum via PE matmuls with
    per-128-edge one-hot selection matrices built on DVE, accumulated in PSUM per
    128-dst tile.  Gather idxs are src-sorted within groups; padding slots are -1
    (skipped by the DMA ucode).
  - dense(l+1) tiles are emitted immediately after each agg(l) tile so layers
    pipeline; AllGather triggers sit between layers' gather streams on gpsimd.
  - Pooling (segment max over graphs), tiny FC and log_softmax run on host.
"""

import sys

sys.path.insert(0, "/opt/trn_rl_repo")

import numpy as np
import ml_dtypes

import concourse.bass as bass
import concourse.bacc as bacc
import concourse.tile as tile
from concourse import mybir
from concourse.bass_utils import run_bass_kernel_spmd
from concourse.masks import make_identity

P = 128
N_NODES = 100000
N_EDGES = 1600000
N_GRAPHS = 64
N_CORES = 8
NODES_PER_CORE = N_NODES // N_CORES          # 12500
NTILES = (NODES_PER_CORE + P - 1) // P       # 98 (last tile 84 rows)
LAST_ROWS = NODES_PER_CORE - (NTILES - 1) * P  # 84
NCHUNK = 4
QT = [25, 25, 25, 23]                        # quarter tile counts
Q_TSTART = [0, 25, 50, 75]
Q_ROWS = [3200, 3200, 3200, 2900]            # per-core rows per quarter
Q_RSTART = [0, 3200, 6400, 9600]
CH_ROWS = [r * N_CORES for r in Q_ROWS]      # chunk rows: 25600,25600,25600,23200
HQ_ROWS = [[1664, 1536], [1664, 1536], [1664, 1536], [1536, 1364]]  # half-quarter rows/core
# AG pieces (q, h); trigger wave = wave containing the piece's last tile
PIECES = [(q, h) for q in range(4) for h in range(2)]
PIECE_WAVE = [3, 6, 9, 12, 15, 18, 21, 24]
WT = 4                                       # dst tiles per wave
F0 = 512
FW = 128                                     # table width (padded)
NCOLS_MM = [128, 64, 32]                     # agg matmul N per layer
MAX_CALL_BLOCKS = 47                         # ~6K idxs per dma_gather call
BB = 32                                      # sel blocks built per DVE instruction
NQUEUES = 4
DMA_SCRATCH = 16384
dt = mybir.dt
BF = ml_dtypes.bfloat16


def _rows(t):
    return LAST_ROWS if t == NTILES - 1 else P


def _host_prep(edge_index):
    """Build the shared (cross-core) aggregation schedule + per-core index data."""
    src = np.concatenate([edge_index[0], np.arange(N_NODES, dtype=np.int64)])
    dst = np.concatenate([edge_index[1], np.arange(N_NODES, dtype=np.int64)])
    deg = np.bincount(dst, minlength=N_NODES).astype(np.float32)
    dinv = (1.0 / np.sqrt(deg)).astype(np.float32)

    waves = [list(range(w, min(w + WT, NTILES))) for w in range(0, NTILES, WT)]

    # src -> (chunk, row-in-chunk): chunk = quarter of the owner's tile range
    s_core = src // NODES_PER_CORE
    s_loc = src % NODES_PER_CORE
    s_t = s_loc // P
    qs = np.searchsorted(np.array([25, 50, 75]), s_t, side="right")  # quarter id
    q_rows = np.array(Q_ROWS)
    q_rstart = np.array(Q_RSTART)
    s_row = s_core * q_rows[qs] + (s_loc - q_rstart[qs])

    per_core = []
    cnts = np.zeros((N_CORES, NTILES, NCHUNK), np.int64)
    d_core = dst // NODES_PER_CORE
    for c in range(N_CORES):
        m = d_core == c
        srow, sq = s_row[m], qs[m]
        dl = dst[m] - c * NODES_PER_CORE
        t = dl // P
        key = t * NCHUNK + sq
        o = np.lexsort((srow, key))          # group by (t,chunk), src-sorted within
        srow, dl, key = srow[o], dl[o], key[o]
        cnt = np.bincount(key, minlength=NTILES * NCHUNK).reshape(NTILES, NCHUNK)
        cnts[c] = cnt
        per_core.append((srow, dl, cnt))

    NW = len(waves)
    # contiguous per-(wave,chunk) slot packing: per-core tile segments are
    # concatenated without per-tile 128-padding; only the (w,ch) region is
    # padded to a block multiple (shared across cores via max).
    wcnt = np.zeros((N_CORES, NW, NCHUNK), np.int64)
    for c in range(N_CORES):
        for w, wtiles in enumerate(waves):
            for ch in range(NCHUNK):
                wcnt[c, w, ch] = cnts[c, wtiles, ch].sum()
    blocks2 = np.maximum((wcnt.max(axis=0) + P - 1) // P, 1)  # [NW, NCHUNK]

    chunk_start = [dict() for _ in range(NCHUNK)]   # [ch][w] -> slot offset
    for ch in range(NCHUNK):
        pos = 0
        for w in range(NW):
            chunk_start[ch][w] = pos
            pos += int(blocks2[w, ch]) * P
    S_ch = [int(blocks2[:, ch].sum() * P) for ch in range(NCHUNK)]

    # union over cores of (tile, block) pairs per (w, ch)
    entry_set = [set() for _ in range(NW)]
    for c in range(N_CORES):
        for w, wtiles in enumerate(waves):
            for ch in range(NCHUNK):
                s = 0
                for t in wtiles:
                    n = int(cnts[c, t, ch])
                    if n:
                        for b in range(s // P, (s + n + P - 1) // P):
                            entry_set[w].add((t, ch, b))
                    s += n
    # didx col order: per wave, sorted by (tile, chunk, block)
    wave_entries = []   # w -> {t: [(gcol, ch, b), ...]}
    wave_gw = []        # w -> (gw0, gw1)
    g = 0
    for w, wtiles in enumerate(waves):
        ents = sorted(entry_set[w])
        gw0 = g
        per_t = {t: [] for t in wtiles}
        for (t, ch, b) in ents:
            per_t[t].append((g, ch, b))
            g += 1
        wave_entries.append(per_t)
        wave_gw.append((gw0, g))
        for t in wtiles:
            assert per_t[t], f"tile {t} has no agg entries"
    NB_total = g

    idx16 = [np.zeros((N_CORES, P, S_ch[ch] // 16), np.int16) for ch in range(NCHUNK)]
    didx = np.full((N_CORES, P, NB_total), -1.0, np.float32)

    for c in range(N_CORES):
        srow, dl, cnt = per_core[c]
        ends = np.cumsum(cnt.reshape(-1))
        starts = ends - cnt.reshape(-1)
        idx_slots = [np.zeros(S_ch[ch], np.int16) for ch in range(NCHUNK)]
        didx_c = np.full((NB_total, P), -1.0, np.float32)
        for w, wtiles in enumerate(waves):
            for ch in range(NCHUNK):
                # concatenated segments for this core
                segs = []
                for t in wtiles:
                    k = t * NCHUNK + ch
                    segs.append((t, starts[k], ends[k]))
                cs = chunk_start[ch][w]
                pos = 0
                bounds = {}
                for t, e0, e1 in segs:
                    n = e1 - e0
                    if n:
                        idx_slots[ch][cs + pos:cs + pos + n] = srow[e0:e1].astype(np.int16)
                        bounds[t] = (pos, pos + n, e0)
                    pos += n
                if pos:
                    # pad region tail with the last valid index (dup descriptor)
                    idx_slots[ch][cs + pos:cs + blocks2[w, ch] * P] = idx_slots[ch][cs + pos - 1]
                # didx columns for this core's (t, block) coverage
                for t in wtiles:
                    if t not in bounds:
                        continue
                    p0, p1, e0 = bounds[t]
                    dloc = (dl[e0:e0 + (p1 - p0)] - t * P).astype(np.float32)
                    for (gcol, ech, b) in wave_entries[w][t]:
                        if ech != ch:
                            continue
                        lo = max(p0, b * P)
                        hi = min(p1, (b + 1) * P)
                        if lo < hi:
                            didx_c[gcol, lo - b * P:hi - b * P] = dloc[lo - p0:hi - p0]
        for ch in range(NCHUNK):
            w16 = idx_slots[ch].reshape(-1, 16).T  # [16, S/16]
            idx16[ch][c] = np.tile(w16, (8, 1))
        didx[c] = didx_c.T

    max_nb = int(blocks2.max())
    meta = {
        "waves": waves,
        "blocks2": blocks2,
        "S_ch": S_ch,
        "NB_total": NB_total,
        "chunk_start": chunk_start,
        "wave_entries": wave_entries,
        "wave_gw": wave_gw,
        "max_nb": max_nb,
    }
    return dinv, idx16, didx, meta


def _build_program(meta, has_bias):
    waves = meta["waves"]
    blocks2 = meta["blocks2"]
    S_ch = meta["S_ch"]
    NB_total = meta["NB_total"]
    chunk_start = meta["chunk_start"]
    wave_entries = meta["wave_entries"]
    wave_gw = meta["wave_gw"]
    max_nb = meta["max_nb"]

    nc = bacc.Bacc(
        "TRN2", target_bir_lowering=False, debug=False, num_devices=N_CORES,
        num_swdge_queues=NQUEUES, dynamic_dma_scratch_size=DMA_SCRATCH,
    )

    xT_io = nc.dram_tensor("xT", [F0, NODES_PER_CORE], dt.bfloat16, kind="ExternalInput").ap()
    sc_in_io = nc.dram_tensor("scIn", [P, NTILES], dt.float32, kind="ExternalInput").ap()   # dinv^2
    sc_out_io = nc.dram_tensor("scOut", [P, NTILES], dt.float32, kind="ExternalInput").ap() # dinv
    w1_io = nc.dram_tensor("W1sb", [P, F0], dt.bfloat16, kind="ExternalInput").ap()
    w2_io = nc.dram_tensor("W2pad", [P, P], dt.bfloat16, kind="ExternalInput").ap()
    w3_io = nc.dram_tensor("W3pad", [P, P], dt.bfloat16, kind="ExternalInput").ap()
    iota_io = nc.dram_tensor("iota", [P, P], dt.bfloat16, kind="ExternalInput").ap()
    iotaR_io = nc.dram_tensor("iotaR", [P, P * BB], dt.bfloat16, kind="ExternalInput").ap()
    idx_ios = [
        nc.dram_tensor(f"idx{ch}", [P, S_ch[ch] // 16], dt.int16, kind="ExternalInput").ap()
        for ch in range(NCHUNK)
    ]
    didx_io = nc.dram_tensor("didx", [P, NB_total], dt.bfloat16, kind="ExternalInput").ap()
    out_io = nc.dram_tensor("out3", [NODES_PER_CORE, 32], dt.float32, kind="ExternalOutput").ap()

    with tile.TileContext(nc) as tc:
        with (
            tc.tile_pool(name="const", bufs=1) as constp,
            tc.tile_pool(name="aT", bufs=1) as aTp,
            tc.tile_pool(name="hs", bufs=8) as hsp,
            tc.tile_pool(name="psum_d", bufs=2, space="PSUM") as psdp,
            tc.tile_pool(name="psum_a", bufs=4, space="PSUM") as psap,
            tc.tile_pool(name="psum_t", bufs=2, space="PSUM") as pstp,
            tc.tile_pool(name="dram", bufs=1, space="DRAM") as dramp,
        ):
            # ---- constants ----
            iota_t = constp.tile([P, P], dt.bfloat16)
            nc.sync.dma_start(iota_t[:], iota_io[:])
            sc_in = constp.tile([P, NTILES], dt.float32)
            nc.sync.dma_start(sc_in[:], sc_in_io[:])
            sc_out = constp.tile([P, NTILES], dt.float32)
            nc.sync.dma_start(sc_out[:], sc_out_io[:])
            w1_sb = constp.tile([P, F0], dt.bfloat16)
            nc.sync.dma_start(w1_sb[:], w1_io[:])
            w2_sb = constp.tile([P, P], dt.bfloat16)
            nc.sync.dma_start(w2_sb[:], w2_io[:])
            w3_sb = constp.tile([P, P], dt.bfloat16)
            nc.sync.dma_start(w3_sb[:], w3_io[:])
            didx_sb = constp.tile([P, NB_total], dt.bfloat16)
            nc.sync.dma_start(didx_sb[:], didx_io[:])
            iotaR = constp.tile([P, P, BB], dt.bfloat16)   # [p, j, g] = j
            nc.sync.dma_start(iotaR[:], iotaR_io[:])
            identb = constp.tile([P, P], dt.bfloat16)
            make_identity(nc, identb[:])

            # a_preT tiles (one set, reallocated per layer): [128 feats, qrows]
            current_aT = {}

            bounces = [
                dramp.tile([NODES_PER_CORE, FW], dt.bfloat16, tag=f"bnc{l}", name=f"bnc{l}")
                for l in range(3)
            ]
            tbls = [
                [dramp.tile([CH_ROWS[q], FW], dt.bfloat16, tag=f"tbl{l}q{q}",
                            name=f"tbl{l}q{q}", addr_space="Shared") for q in range(4)]
                for l in range(3)
            ]

            def q_of_t(t):
                return min(t // 25, 3)

            def dense_tile(l, t):
                """hs tile = a_pre @ W  (psum) -> bf16 hs tile -> bounce DRAM."""
                r = _rows(t)
                c0 = t * P
                ps = psdp.tile([P, P], dt.float32, space="PSUM", tag="pd")
                if l == 0:
                    h = 0 if t < 49 else 1
                    hb = 0 if t < 49 else 49 * P
                    nk = F0 // P
                    for k in range(nk):
                        xk = xhalves[h][k]
                        nc.tensor.matmul(
                            out=ps[:r, :], lhsT=xk[:, c0 - hb:c0 - hb + r],
                            rhs=w1_sb[:, k * P:(k + 1) * P],
                            start=(k == 0), stop=(k == nk - 1),
                        )
                else:
                    q = q_of_t(t)
                    off = (t - Q_TSTART[q]) * P
                    w = w2_sb if l == 1 else w3_sb
                    nc.tensor.matmul(
                        out=ps[:r, :], lhsT=current_aT[l][q][:, off:off + r], rhs=w[:],
                        start=True, stop=True,
                    )
                hs = hsp.tile([P, P], dt.bfloat16, tag="hs")
                nc.vector.tensor_copy(hs[:r, :], ps[:r, :])
                nc.scalar.dma_start(bounces[l][c0:c0 + r, :], hs[:r, :])

            def emit_ag(l, q):
                r0 = Q_RSTART[q]
                nc.gpsimd.collective_compute(
                    "AllGather", mybir.AluOpType.bypass,
                    replica_groups=[list(range(N_CORES))],
                    ins=[bounces[l][r0:r0 + Q_ROWS[q], :]],
                    outs=[tbls[l][q][:]],
                )

            # ---- dense layer 0: staged x stripes, two node-halves ----
            xhalves = []
            with tc.tile_pool(name="xs", bufs=5) as xsp:
                for h in range(2):
                    hb = 0 if h == 0 else 49 * P
                    hc = 49 * P if h == 0 else NODES_PER_CORE - 49 * P
                    ks = []
                    for k in range(F0 // P):
                        xk = xsp.tile([P, 49 * P], dt.bfloat16, tag="xh")
                        eng = nc.scalar if k % 2 == 0 else nc.sync
                        eng.dma_start(
                            xk[:, :hc], xT_io[k * P:(k + 1) * P, hb:hb + hc]
                        )
                        ks.append(xk)
                    xhalves.append(ks)
                for t in range(NTILES):
                    dense_tile(0, t)
                for q in range(3):
                    emit_ag(0, q)

            def agg_layer(l, idxp, msgp, selp, workp):
                ncols = NCOLS_MM[l]
                if l < 2:
                    current_aT[l + 1] = [
                        aTp.tile([P, Q_ROWS[q]], dt.bfloat16, tag=f"aq{q}",
                                 name=f"a{l + 1}q{q}")
                        for q in range(4)
                    ]
                # gather schedule: first 3 waves' ch3 deferred behind their
                # ch0-2 so the last AllGather's latency is covered with work
                NW = len(waves)
                sched = []
                defer = []
                for w in range(NW):
                    if w < 3:
                        sched += [(w, ch) for ch in range(NCHUNK - 1)]
                        defer.append((w, NCHUNK - 1))
                        if w == 2:
                            sched += defer
                    else:
                        sched += [(w, ch) for ch in range(NCHUNK)]
                msg_tiles_all = {}
                sp = [0]  # pointer into sched
                l0_late = [l == 0]  # layer-0 pieces 6-7 emitted at first ch3 gather

                def emit_gathers_due(wlimit):
                    while sp[0] < len(sched) and sched[sp[0]][0] <= wlimit:
                        w, ch = sched[sp[0]]
                        sp[0] += 1
                        if l0_late[0] and ch == NCHUNK - 1:
                            emit_ag(0, 3)
                            l0_late[0] = False
                        nb = int(blocks2[w, ch])
                        s0 = chunk_start[ch][w]
                        S = nb * P
                        iw = idxp.tile([P, max_nb * P // 16], dt.int16, tag="idx")
                        nc.sync.dma_start(iw[:, :S // 16], idx_ios[ch][:, s0 // 16:(s0 + S) // 16])
                        mt = msgp.tile([P, max_nb, FW], dt.bfloat16, tag="msg")
                        for b0 in range(0, nb, MAX_CALL_BLOCKS):
                            b1 = min(b0 + MAX_CALL_BLOCKS, nb)
                            Ssub = (b1 - b0) * P
                            nc.gpsimd.dma_gather(
                                out_ap=mt[:, b0:b1, :],
                                in_ap=tbls[l][ch][:],
                                idxs_ap=iw[:, b0 * P // 16:b1 * P // 16],
                                num_idxs=Ssub, num_idxs_reg=Ssub,
                                elem_size=FW, elem_step=FW,
                                single_packet=False,
                                queue_num=ch % NQUEUES,
                            )
                        msg_tiles_all[(w, ch)] = mt

                # quarter q of the next layer is fully produced after this wave
                qend = {6: 0, 12: 1, 18: 2} if l < 2 else {}
                for w, wtiles in enumerate(waves):
                    emit_gathers_due(w + 3)
                    # sel blocks: [slot, dst, block] layout so every operand's
                    # last dim is stride-1 (enables the DVE 2x mode)
                    gw0, gw1 = wave_gw[w]
                    sel_tiles = {}
                    for q0 in range(gw0, gw1, BB):
                        q1 = min(q0 + BB, gw1)
                        st = selp.tile([P, P, BB], dt.bfloat16, tag="sel")
                        nc.vector.tensor_tensor(
                            out=st[:, :, :q1 - q0],
                            in0=didx_sb[:, None, q0:q1].to_broadcast([P, P, q1 - q0]),
                            in1=iotaR[:, :, :q1 - q0],
                            op=mybir.AluOpType.is_equal,
                        )
                        for q in range(q0, q1):
                            sel_tiles[q] = (st, q - q0)

                    # per-tile matmul accumulation + post + (dense l+1)
                    for ti, t in enumerate(wtiles):
                        r = _rows(t)
                        c0 = t * P
                        ps = psap.tile([P, P], dt.float32, space="PSUM", tag="pa")
                        mms = [(gcol, ch, b) for (gcol, ch, b) in wave_entries[w][t]]
                        for i, (q, ch, col) in enumerate(mms):
                            st, j = sel_tiles[q]
                            nc.tensor.matmul(
                                out=ps[:r, :ncols], lhsT=st[:, :r, j],
                                rhs=msg_tiles_all[(w, ch)][:, col, :ncols],
                                start=(i == 0),
                                stop=(i == len(mms) - 1),
                            )
                        if l < 2:
                            outt = workp.tile([P, P], dt.bfloat16, tag="outb")
                            nc.scalar.activation(
                                outt[:r, :], ps[:r, :],
                                mybir.ActivationFunctionType.Relu,
                                scale=sc_in[:r, t:t + 1],
                            )
                            pst = pstp.tile([P, P], dt.bfloat16, space="PSUM", tag="pt")
                            nc.tensor.transpose(
                                out=pst[:, :r], in_=outt[:r, :], identity=identb[:r, :r]
                            )
                            q = q_of_t(t)
                            off = (t - Q_TSTART[q]) * P
                            nc.scalar.activation(
                                current_aT[l + 1][q][:, off:off + r], pst[:, :r],
                                mybir.ActivationFunctionType.Copy,
                            )
                            dense_tile(l + 1, t)
                        else:
                            outt = workp.tile([P, 32], dt.float32, tag="outf")
                            nc.scalar.activation(
                                outt[:r, :], ps[:r, :32],
                                mybir.ActivationFunctionType.Relu,
                                scale=sc_out[:r, t:t + 1],
                            )
                            nc.scalar.dma_start(out_io[c0:c0 + r, :], outt[:r, :])
                    if w in qend:
                        emit_ag(l + 1, qend[w])
                if l < 2:
                    emit_ag(l + 1, 3)

            with (
                tc.tile_pool(name="idxw", bufs=20) as idxp,
                tc.tile_pool(name="msgs", bufs=16) as msgp,
                tc.tile_pool(name="sel", bufs=5) as selp,
                tc.tile_pool(name="work", bufs=6) as workp,
            ):
                agg_layer(0, idxp, msgp, selp, workp)
                agg_layer(1, idxp, msgp, selp, workp)
                agg_layer(2, idxp, msgp, selp, workp)

    nc.compile()
    return nc


def _pack_inputs(x, dinv, W1, W2, W3, idx16, didx):
    iota_rep = np.tile(np.arange(P, dtype=np.float32)[None, :], (P, 1)).astype(BF)
    iotaR = np.tile(
        np.repeat(np.arange(P, dtype=np.float32), BB)[None, :], (P, 1)
    ).astype(BF)  # [p, j*BB + g] = j
    w1sb = np.zeros((P, F0), np.float32)
    for k in range(F0 // P):
        w1sb[:, k * P:(k + 1) * P] = W1[k * P:(k + 1) * P, :]
    w2pad = np.zeros((P, P), np.float32)
    w2pad[:, :64] = W2
    w3pad = np.zeros((P, P), np.float32)
    w3pad[:64, :32] = W3

    in_maps = []
    for c in range(N_CORES):
        lo = c * NODES_PER_CORE
        dv = dinv[lo:lo + NODES_PER_CORE]
        xs = x[lo:lo + NODES_PER_CORE].astype(np.float32) * dv[:, None]
        sc_in = np.ones((P, NTILES), np.float32)   # dinv^2 per tile col
        sc_out = np.ones((P, NTILES), np.float32)  # dinv per tile col
        for t in range(NTILES):
            r = _rows(t)
            sc_in[:r, t] = dv[t * P:t * P + r] ** 2
            sc_out[:r, t] = dv[t * P:t * P + r]
        in_maps.append({
            "xT": np.ascontiguousarray(xs.T).astype(BF),
            "scIn": sc_in,
            "scOut": sc_out,
            "W1sb": w1sb.astype(BF),
            "W2pad": w2pad.astype(BF),
            "W3pad": w3pad.astype(BF),
            "iota": iota_rep,
            "iotaR": iotaR,
            **{f"idx{ch}": idx16[ch][c] for ch in range(NCHUNK)},
            "didx": didx[c].astype(BF),
        })
    return in_maps


_TRACE = [False]          # set by test harness to request a profiled run
_LAST_RESULT = [None]     # BassKernelResults of the last run (for profiling)


def kernel(x, edge_index, batch, W1, b1, W2, b2, W3, b3, Wfc, bfc):
    x = np.asarray(x)
    edge_index = np.asarray(edge_index)
    batch = np.asarray(batch)
    W1, b1 = np.asarray(W1), np.asarray(b1)
    W2, b2 = np.asarray(W2), np.asarray(b2)
    W3, b3 = np.asarray(W3), np.asarray(b3)
    Wfc, bfc = np.asarray(Wfc), np.asarray(bfc)
    has_bias = bool(np.abs(b1).max() or np.abs(b2).max() or np.abs(b3).max())
    assert not has_bias, "nonzero conv biases not wired up in v2"

    dinv, idx16, didx, meta = _host_prep(edge_index.astype(np.int64))
    nc = _build_program(meta, has_bias)
    in_maps = _pack_inputs(x, dinv, W1, W2, W3, idx16, didx)
    res = run_bass_kernel_spmd(
        nc, in_maps, core_ids=list(range(N_CORES)), trace=_TRACE[0]
    )
    _LAST_RESULT[0] = res

    h3 = np.concatenate([res.results[c]["out3"][:, :32] for c in range(N_CORES)], axis=0)

    # host epilogue: segment max pool + FC + log_softmax (float64 for stability)
    pooled = np.full((N_GRAPHS, 32), -np.inf, np.float64)
    bnd = np.searchsorted(batch, np.arange(N_GRAPHS + 1))
    for g in range(N_GRAPHS):
        if bnd[g + 1] > bnd[g]:
            pooled[g] = h3[bnd[g]:bnd[g + 1]].max(axis=0)
    logits = pooled @ Wfc.astype(np.float64) + bfc.astype(np.float64)
    m = logits.max(axis=1, keepdims=True)
    lse = m + np.log(np.exp(logits - m).sum(axis=1, keepdims=True))
    return (logits - lse).astype(np.float32)
